# revision 1
# baseline (speedup 1.0000x reference)
"""Trainium2 Bass kernel for sparse (causal, tanh-clamped) attention.

Problem: B=2, L=2048, D=1024, H=16 heads x 64 dim; S = QK^T/8;
S = 30*tanh(S); causal + attention_mask; softmax; out = attn @ V.

Sharding: 2 heads per core across 8 cores (tensor-parallel on heads).
Each core computes its 128 output features for the full batch.

Key design points:
 - All matmuls run in float32r (TF32-like, 1 cyc/row on PE for moving
   dim >= 256; HW rounds fp32 inputs internally).
 - Everything is computed in the transposed layout S^T[k, q] so that no
   P-matrix transpose is needed: S^T = K_aug^T @ Q_aug with the
   contraction (d) on partitions; the softmax numerator P^T feeds the
   AV matmul directly as the moving operand.
 - attention_mask is folded into the score matmul via an augmented 65th
   contraction row: K row 64 = (mask-1)*1e6, Q row 64 = 1.  tanh then
   saturates masked scores to -1 -> P = e^-60 ~ 0.
 - Bounded logits (30*tanh in [-30, 30]) mean softmax needs no running
   max: P = exp(30*tanh(s) - 30) in (0, 1]; the denominator comes for
   free as a ones-column appended to V in the AV matmul.
 - Causal masking: per k-tile the q range starts at the diagonal; only
   the 128x128 diagonal block needs a triu multiply on P.
 - ACT (tanh+exp, the bottleneck engine) runs on wide strips (up to
   1024 columns); tanh is computed in place in PSUM (cheaper ACT
   access).  Projections pack Q|K|V for a 256-token chunk into a
   single 2-bank PSUM slot so they interleave with attention instead
   of starving it; batch 0's attention overlaps batch 1's projections.
"""

import sys

if "/opt/trn_rl_repo" not in sys.path:
    sys.path.insert(0, "/opt/trn_rl_repo")

import numpy as np

B = 2
L = 2048
D = 1024
H = 16
DH = 64
N_CORES = 8
T = B * L            # 4096 tokens
E = 128              # per-core output features (2 heads)
NEG_BIG = 1.0e6      # mask additive; tanh saturates anything big
TAU = 30.0

_CACHE = {}


def _build_program():
    import concourse.bacc as bacc
    import concourse.tile as tile
    from concourse import mybir

    F32 = mybir.dt.float32
    F32R = mybir.dt.float32r
    AF = mybir.ActivationFunctionType

    nc = bacc.Bacc("TRN2", target_bir_lowering=False, debug=False,
                   num_devices=N_CORES)

    xT_d = nc.dram_tensor("xT", [D, T], F32R, kind="ExternalInput")
    wq_d = nc.dram_tensor("wq", [D, E], F32R, kind="ExternalInput")
    wk_d = nc.dram_tensor("wk", [D, E], F32R, kind="ExternalInput")
    wv_d = nc.dram_tensor("wv", [D, E], F32R, kind="ExternalInput")
    kaug_d = nc.dram_tensor("kaug", [1, T], F32R, kind="ExternalInput")
    ones_d = nc.dram_tensor("onesrow", [1, T], F32R, kind="ExternalInput")
    onescol_d = nc.dram_tensor("onescol", [128, 1], F32R, kind="ExternalInput")
    tril_d = nc.dram_tensor("tril", [128, 128], F32, kind="ExternalInput")
    ident_d = nc.dram_tensor("ident", [128, 128], F32R, kind="ExternalInput")
    out_d = nc.dram_tensor("out", [B, L, E], F32, kind="ExternalOutput")

    ND = D // 128        # 8 contraction chunks for projections
    NTB = L // 512       # 4 512-token groups per batch
    NK = L // 128        # 16 k tiles per sequence
    QH = 1024            # attention q-half width

    with tile.TileContext(nc) as tc:
        with (
            tc.tile_pool(name="const", bufs=1) as constp,
            tc.tile_pool(name="weights", bufs=1) as wp,
            tc.tile_pool(name="qkv", bufs=1) as qkvp,
            tc.tile_pool(name="xin", bufs=12) as xp,
            tc.tile_pool(name="work", bufs=3) as workp,
            tc.tile_pool(name="vaug", bufs=36) as vaugp,
            tc.tile_pool(name="epi", bufs=3) as epip,
            tc.tile_pool(name="ostage", bufs=32) as ostagep,
            tc.tile_pool(name="strip", bufs=3, space="PSUM") as stripp,
            tc.tile_pool(name="psO", bufs=1, space="PSUM") as psOp,
        ):
            tril_t = constp.tile([128, 128], F32, tag="tril")
            ident_t = constp.tile([128, 128], F32R, tag="ident")
            onescol_t = constp.tile([128, 1], F32R, tag="onescol")
            n30_t = constp.tile([128, 1], F32, tag="n30")
            nc.gpsimd.memset(n30_t[:], -TAU)
            identf_t = constp.tile([128, 128], F32, tag="identf")

            # weight tiles: w[:, d*128:(d+1)*128] = W.T chunk d ([128, 128])
            w_tiles = []
            for name, d_in in (("wq", wq_d), ("wk", wk_d), ("wv", wv_d)):
                wt = wp.tile([128, ND * E], F32R, tag=name, name=name)
                nc.sync.dma_start(
                    wt[:].rearrange("p (d e) -> p d e", d=ND),
                    d_in.ap().rearrange("(d p) e -> p d e", p=128),
                )
                w_tiles.append(wt)
            nc.sync.dma_start(ident_t[:], ident_d.ap()[:])
            nc.sync.dma_start(tril_t[:], tril_d.ap()[:])

            # Per (head, batch) QKV storage; row 64 = augmentation row.
            QT = [[qkvp.tile([65, L], F32R, tag=f"qt{h}{b}", name=f"qt{h}{b}")
                   for b in range(B)] for h in range(2)]
            KT = [[qkvp.tile([65, L], F32R, tag=f"kt{h}{b}", name=f"kt{h}{b}")
                   for b in range(B)] for h in range(2)]
            VT = [[qkvp.tile([64, L], F32R, tag=f"vt{h}{b}", name=f"vt{h}{b}")
                   for b in range(B)] for h in range(2)]
            def load_aug_rows(h, b):
                sl = slice(b * L, (b + 1) * L)
                nc.sync.dma_start(QT[h][b][64:65, :], ones_d.ap()[0:1, sl])
                nc.sync.dma_start(KT[h][b][64:65, :], kaug_d.ap()[0:1, sl])

            def project_group_loads(b, tp):
                g0 = b * L + tp * 512
                xts = []
                for d in range(ND):
                    xt = xp.tile([128, 512], F32R, tag="xt", name="xt")
                    nc.sync.dma_start(
                        xt[:], xT_d.ap()[d * 128:(d + 1) * 128,
                                         g0:g0 + 512])
                    xts.append(xt)
                return xts

            def project_group(b, tp, act_drains=False):
                """QKV projections for one 512-token group of batch b.

                Q|K|V for a 256-token chunk pack into ONE 2-bank strip
                slot, so a projection in flight holds a single PSUM
                slot and can interleave with attention.  Drains go to
                ACT when it is known-idle (prologue), else DVE.
                """
                xts = project_group_loads(b, tp)
                for half in (0, 1):
                    project_group_half(b, tp, xts, half, act_drains)

            def project_group_half(b, tp, xts, half, act_drains=False,
                                    ps=(0, 1, 2), pj=None):
                c0 = half * 256
                t0 = tp * 512 + c0
                if pj is None:
                    pj = stripp.tile([128, 1024], F32, tag="strip",
                                     name="pj")
                for d in range(ND):
                    for p in ps:
                        # start marks a whole 2KB PSUM zero-region as
                        # pending-zero; Q (p=0) and K (p=1) share bank
                        # 0, so only Q sets start or K's start would
                        # wipe Q's partials.  K's first-touch bytes are
                        # pending-zero from Q's mark and zero-fill.
                        nc.tensor.matmul(
                            pj[:, p * 256:p * 256 + 256],
                            w_tiles[p][:, d * E:(d + 1) * E],
                            xts[d][:, c0:c0 + 256],
                            start=(d == 0 and p != 1),
                            stop=(d == ND - 1),
                        )
                dsts = (QT, KT, VT)
                for h in range(2):
                    sl = slice(h * 64, h * 64 + 64)
                    ts_ = slice(t0, t0 + 256)
                    for p in ps:
                        dst = dsts[p]
                        csl = slice(p * 256, p * 256 + 256)
                        if act_drains and dst is not VT:
                            nc.scalar.activation(dst[h][b][0:64, ts_],
                                                 pj[sl, csl],
                                                 AF.Identity)
                        else:
                            nc.vector.tensor_copy(dst[h][b][0:64, ts_],
                                                  pj[sl, csl])
                return pj

            def vaug_prologue(b, h, kis):
                """V^T -> V tiles for one unit, with a ones column."""
                vaug = []
                for ki in kis:
                    pvt = stripp.tile([128, 64], F32R, tag="strip",
                                      name="pvt")
                    nc.tensor.transpose(
                        pvt[:], VT[h][b][0:64, ki * 128:ki * 128 + 128],
                        ident_t[0:64, 0:64])
                    va = vaugp.tile([128, 65], F32R, tag="vaug", name="va")
                    nc.vector.tensor_copy(va[:, 0:64], pvt[:])
                    nc.vector.tensor_copy(va[:, 64:65], onescol_t[:])
                    vaug.append(va)
                return vaug

            def attention_span(b, h, qlo, qw, vaug, ostage, pump):
                """Causal attention for q in [qlo, qlo+qw) of one (b, h)
                unit (qw = 512 or 1024, 512-aligned).

                `pump()` emits one queued background work unit (a
                projection piece or V prologue for a later unit); it is
                called once per k-tile so PE/DVE fill gaps while ACT
                stays busy.
                """
                po = psOp.tile([65, qw], F32, tag="psO", name="po")
                epilogue_half = make_epilogue(b, h, qlo, po, ostage)
                kimax = (qlo + qw) // 128 - 1
                for ki in range(kimax + 1):
                    q0 = max(qlo, ki * 128)
                    w = qlo + qw - q0
                    pss = stripp.tile([128, QH], F32, tag="strip",
                                      name="pss")
                    for off in range(0, w, 512):
                        ln = min(512, w - off)
                        nc.tensor.matmul(
                            pss[:, off:off + ln],
                            KT[h][b][:, ki * 128:ki * 128 + 128],
                            QT[h][b][:, q0 + off:q0 + off + ln],
                            start=True, stop=True)
                    # tanh in place in PSUM, then exp -> SBUF f32r
                    nc.scalar.activation(pss[:, 0:w], pss[:, 0:w],
                                         AF.Tanh, scale=0.125)
                    pp = workp.tile([128, QH], F32R, tag="prob",
                                    name="pp", bufs=6)
                    nc.scalar.activation(pp[:, 0:w], pss[:, 0:w],
                                         AF.Exp, bias=n30_t[:],
                                         scale=TAU)
                    if ki * 128 >= qlo:
                        nc.vector.tensor_mul(pp[:, 0:128], pp[:, 0:128],
                                             tril_t[:])
                    # accumulate AV per 512-wide q chunk
                    for qc in range(qlo // 512, (qlo + qw) // 512):
                        c0 = qc * 512
                        if c0 + 512 <= q0:
                            continue
                        a0 = max(q0, c0)
                        ln = c0 + 512 - a0
                        nc.tensor.matmul(
                            po[:, a0 - qlo:a0 - qlo + ln],
                            vaug[ki][:],
                            pp[:, a0 - q0:a0 - q0 + ln],
                            start=(ki == 0),
                            stop=(ki == min(kimax, 4 * qc + 3)))
                    pump()
                    if qw == 1024 and ki == 4 * (qlo // 512) + 3:
                        # first 512 columns of po are final: drain them
                        # while the remaining k-tiles accumulate the rest
                        epilogue_half(0)
                if qw == 1024:
                    epilogue_half(1)
                else:
                    epilogue_half(0)

            def make_epilogue(b, h, qlo, po, ostage):
                def epilogue_half(half):
                    # transpose back, normalize, store (one 512 chunk)
                    e0 = half * 512
                    ot = epip.tile([65, 512], F32, tag="ot", name="ot")
                    nc.vector.tensor_copy(ot[:], po[:, e0:e0 + 512])
                    for j in range(4):
                        qt_ = (qlo + e0 + j * 128) // 128  # global q tile
                        pt = psOp.tile([128, 65], F32, tag="psO",
                                       name="pt")
                        nc.tensor.transpose(
                            pt[:], ot[:, j * 128:(j + 1) * 128],
                            identf_t[0:65, 0:65])
                        of = epip.tile([128, 65], F32, tag="of", name="of")
                        nc.vector.tensor_copy(of[:], pt[:])
                        rec = epip.tile([128, 1], F32, tag="rec",
                                        name="rec")
                        nc.vector.reciprocal(rec[:], of[:, 64:65])
                        nc.vector.tensor_scalar_mul(
                            ostage[qt_][:, h * 64:(h + 1) * 64],
                            of[:, 0:64], rec[:])
                        if h == 1:   # both heads done -> store
                            nc.gpsimd.dma_start(
                                out_d.ap()[b, qt_ * 128:(qt_ + 1) * 128, :],
                                ostage[qt_][:])
                return epilogue_half

            ostages = [[ostagep.tile([128, 128], F32, tag="ostage",
                                     name=f"os{b}_{j}")
                        for j in range(L // 128)] for b in range(B)]

            # Orchestration: emit the minimum prologue directly, queue the
            # rest as background units pumped from inside the attention
            # loops (one unit per two pump points to spread PE load).
            from collections import deque
            pending = deque()

            def pump():
                if pending:
                    pending.popleft()()

            def flush():
                while pending:
                    pending.popleft()()

            vaugs = {}

            def queue_vaug(b, h, kis):
                def unit():
                    vaugs.setdefault((b, h), []).extend(
                        vaug_prologue(b, h, kis))
                return unit

            def queue_proj(b, tp):
                """Two pump units per 512-group (finer PE granularity)."""
                shared = {}

                def unit0():
                    shared["x"] = project_group_loads(b, tp)
                    project_group_half(b, tp, shared["x"], 0)

                def unit1():
                    project_group_half(b, tp, shared["x"], 1)
                return [unit0, unit1]

            # tokens 0:512 of batch 0 project first, drains on idle
            # ACT; the first 512-wide attention span starts right after.
            xts00 = project_group_loads(0, 0)
            load_aug_rows(0, 0)
            nc.sync.dma_start(onescol_t[:], onescol_d.ap()[:])
            nc.sync.dma_start(identf_t[:].bitcast(F32R), ident_d.ap()[:])
            load_aug_rows(1, 0)
            for half in (0, 1):
                project_group_half(0, 0, xts00, half, act_drains=True)
            vaugs[(0, 0)] = vaug_prologue(0, 0, range(4))
            load_aug_rows(0, 1)
            load_aug_rows(1, 1)

            pending.extend(queue_proj(0, 1))
            pending.append(queue_vaug(0, 0, range(4, 8)))
            attention_span(0, 0, 0, 512, vaugs[(0, 0)], ostages[0], pump)
            flush()
            pending.extend(queue_proj(0, 2))
            pending.extend(queue_proj(0, 3))
            pending.append(queue_vaug(0, 0, range(8, 12)))
            pending.append(queue_vaug(0, 0, range(12, NK)))
            attention_span(0, 0, 512, 512, vaugs[(0, 0)], ostages[0], pump)
            spans = [
                (0, 0, 1), (0, 1, 0), (0, 1, 1),
                (1, 0, 0), (1, 0, 1), (1, 1, 0), (1, 1, 1),
            ]
            hooks = {
                0: [queue_vaug(0, 1, range(0, 8)),
                    queue_vaug(0, 1, range(8, NK))]
                   + [u for tp in range(NTB) for u in queue_proj(1, tp)],
                2: [queue_vaug(1, 0, range(0, 8)),
                    queue_vaug(1, 0, range(8, NK))],
                4: [queue_vaug(1, 1, range(0, 8)),
                    queue_vaug(1, 1, range(8, NK))],
            }
            flush_before = {0: True, 3: True, 5: True}
            for i, (b, h, qh) in enumerate(spans):
                if flush_before.get(i):
                    flush()
                for u in hooks.get(i, []):
                    pending.append(u)
                attention_span(b, h, qh * QH, QH, vaugs[(b, h)],
                               ostages[b], pump)
            flush()

    nc.compile()
    return nc


def _get_program():
    if "nc" not in _CACHE:
        _CACHE["nc"] = _build_program()
    return _CACHE["nc"]


def _prep_inputs(input, attention_mask, W_Q, W_K, W_V):
    x = np.asarray(input, dtype=np.float32).reshape(T, D)
    xT = np.ascontiguousarray(x.T)                          # [D, T]
    mask = np.asarray(attention_mask).astype(np.float32).reshape(1, T)
    kaug = (mask - 1.0) * NEG_BIG                           # 0 keep, -1e6 drop
    onesrow = np.ones((1, T), dtype=np.float32)
    onescol = np.ones((128, 1), dtype=np.float32)
    tril = np.triu(np.ones((128, 128), dtype=np.float32))   # keep[k, q] = q >= k
    ident = np.eye(128, dtype=np.float32)

    common = {
        "xT": xT, "kaug": kaug, "onesrow": onesrow, "onescol": onescol,
        "tril": tril, "ident": ident,
    }
    in_maps = []
    for c in range(N_CORES):
        sl = slice(c * E, (c + 1) * E)
        in_maps.append({
            **common,
            "wq": np.ascontiguousarray(np.asarray(W_Q, np.float32)[sl, :].T),
            "wk": np.ascontiguousarray(np.asarray(W_K, np.float32)[sl, :].T),
            "wv": np.ascontiguousarray(np.asarray(W_V, np.float32)[sl, :].T),
        })
    return in_maps


def kernel(input, attention_mask, W_Q, W_K, W_V):
    from concourse.bass_utils import run_bass_kernel_spmd

    nc = _get_program()
    in_maps = _prep_inputs(input, attention_mask, W_Q, W_K, W_V)
    res = run_bass_kernel_spmd(nc, in_maps, list(range(N_CORES)))
    return np.concatenate([res.results[c]["out"] for c in range(N_CORES)],
                          axis=2)



# revision 33
# speedup vs baseline: 1.1673x; 1.1673x over previous
"""Trainium2 Bass kernel for sparse (causal, tanh-clamped) attention.

Problem: B=2, L=2048, D=1024, H=16 heads x 64 dim; S = QK^T/8;
S = 30*tanh(S); causal + attention_mask; softmax; out = attn @ V.

Sharding: 2 heads per core across 8 cores (tensor-parallel on heads).

v2 design (ACT is the bottleneck engine: 2 passes tanh+exp over all
causal score columns ~= 116us + per-instruction overheads ~= 150us):
 - bf16 inputs/weights/QKV/probabilities (matmul cost is per moving row
   regardless of dtype>=bf16; halves DMA bytes and removes the fp32r
   <256-row 4x penalty).
 - S^T[k, q] layout throughout; no P transpose (P^T feeds AV directly).
 - attention_mask folded into the score matmul via 65th contraction row.
 - causal diagonal-block mask applied INSIDE the score accumulation as
   one extra matmul (maskT stationary, identity moving): no DVE/ACT cost.
 - bounded logits: P = exp(30*tanh(s)-30), no running max; denominator
   via a ones-column in V (column 0 of the augmented V tile).
 - V is projected token-major directly (stationary=x chunk, moving=W_V
   chunk) -- no V transposes, no separate vaug copies.
 - AV accumulation per 128-column tile with stop at ki==j: epilogues
   drain per-512 bank as soon as its last k-tile lands (spread through
   the run; tiny tail).
 - software-pipelined emission: per k-tile iteration emit score(ki),
   tanh/exp(ki), then AV(ki-1) and budget-limited background quanta
   (projections/V/stores) so the in-order PE queue never starves ACT.
"""

import sys

if "/opt/trn_rl_repo" not in sys.path:
    sys.path.insert(0, "/opt/trn_rl_repo")

import numpy as np

B = 2
L = 2048
D = 1024
H = 16
DH = 64
N_CORES = 8
T = B * L            # 4096 tokens
E = 128              # per-core output features (2 heads)
ND = D // 128        # 8 contraction chunks
NEG_BIG = 6.0e4   # fp16-safe; still saturates tanh
TAU = 30.0

_CACHE = {}


def _build_program():
    import concourse.bacc as bacc
    import concourse.tile as tile
    from concourse import mybir
    from collections import deque

    F32 = mybir.dt.float32
    F16 = mybir.dt.float16
    BF16 = mybir.dt.bfloat16
    AF = mybir.ActivationFunctionType

    nc = bacc.Bacc("TRN2", target_bir_lowering=False, debug=False,
                   num_devices=N_CORES)

    # --- DRAM tensors -----------------------------------------------------
    # x host-packed: group0 of each batch split in two 256-token pieces
    # laid out [p, (d, t)], remaining groups [p, (g, d, t512)].
    xh0a_d = [nc.dram_tensor(f"xh0a{b}", [128, ND * 256], F16,
                             kind="ExternalInput") for b in range(B)]
    xh0b_d = [nc.dram_tensor(f"xh0b{b}", [128, ND * 256], F16,
                             kind="ExternalInput") for b in range(B)]
    xhr_d = nc.dram_tensor("xhr", [128, 6 * ND * 512], F16,
                           kind="ExternalInput")
    wqk_d = nc.dram_tensor("wqk", [128, 2 * ND * 128], F16,
                           kind="ExternalInput")
    wv_d = nc.dram_tensor("wv", [128, ND * 128], F16,
                          kind="ExternalInput")
    kaug_d = nc.dram_tensor("kaug", [1, T], F16, kind="ExternalInput")
    trilb_d = nc.dram_tensor("trilb", [128, 128], BF16,
                             kind="ExternalInput")
    identf_d = nc.dram_tensor("identf", [128, 128], F32,
                              kind="ExternalInput")
    out_d = nc.dram_tensor("out", [B, L, E], F32, kind="ExternalOutput")

    NJ = L // 128     # 16 column tiles per sequence

    with tile.TileContext(nc) as tc:
        with (
            tc.tile_pool(name="const", bufs=1) as constp,
            tc.tile_pool(name="weights", bufs=1) as wp,
            tc.tile_pool(name="qk", bufs=1) as qkp,
            tc.tile_pool(name="va", bufs=1) as vap,
            tc.tile_pool(name="x0", bufs=4) as x0p,
            tc.tile_pool(name="xg", bufs=6) as xgp,
            tc.tile_pool(name="prob", bufs=6) as ppp,
            tc.tile_pool(name="epi", bufs=6) as epip,
            tc.tile_pool(name="ostage", bufs=1) as ostagep,
            tc.tile_pool(name="strip", bufs=3, space="PSUM") as stripp,
            tc.tile_pool(name="psO", bufs=1, space="PSUM") as pop,
        ):
            # --- constants ----------------------------------------------
            trilb_t = constp.tile([128, 128], BF16, tag="trilb")
            identf_t = constp.tile([128, 128], F32, tag="identf")
            n30_t = constp.tile([128, 1], F32, tag="n30")
            nc.gpsimd.memset(n30_t[:], -TAU)
            wz_t = constp.tile([128, 128], F16, tag="wzero")
            nc.gpsimd.memset(wz_t[:], 0.0)

            wqk_t = wp.tile([128, 2 * ND * 128], F16, tag="wqk",
                            name="wqk")
            wv_t = wp.tile([128, ND * 128], F16, tag="wv", name="wv")

            QT = [[qkp.tile([65, L], F16, tag=f"qt{h}{b}",
                            name=f"qt{h}{b}")
                   for b in range(B)] for h in range(2)]
            KT = [[qkp.tile([65, L], F16, tag=f"kt{h}{b}",
                            name=f"kt{h}{b}")
                   for b in range(B)] for h in range(2)]
            # va[b][ki]: [128 tokens, 131]: col0=ones, 1:65=h0 feats,
            # col65=ones, 66:130=h1 feats
            VA = [[vap.tile([128, 131], BF16, tag=f"va{b}_{k}",
                            name=f"va{b}_{k}") for k in range(NJ)]
                  for b in range(B)]
            OST = [[ostagep.tile([128, 512], F32, tag=f"os{b}_{k}",
                                 name=f"os{b}_{k}") for k in range(4)]
                   for b in range(B)]

            # ones columns of VA (written once; Pool is idle)
            for b in range(B):
                for k in range(NJ):
                    nc.gpsimd.memset(VA[b][k][:, 0:1], 1.0)
                    nc.gpsimd.memset(VA[b][k][:, 65:66], 1.0)

            # --- initial DMAs. The startup chain is HWDGE-bound
            # (~625ns per dma_start on a serial device), so the minimum
            # number of transfers gates the first projection.
            x0_tiles = {}   # (b, half) -> tile
            x0_tiles[(0, 0)] = x0p.tile([128, ND * 256], F16, tag="x0",
                                        name="x00")
            nc.sync.dma_start(x0_tiles[(0, 0)][:], xh0a_d[0].ap()[:])
            nc.sync.dma_start(wqk_t[:, 0:1024], wqk_d.ap()[:, 0:1024])
            nc.sync.dma_start(wqk_t[:, 1024:2048],
                              wqk_d.ap()[:, 1024:2048])
            nc.sync.dma_start(trilb_t[:], trilb_d.ap()[:])
            for h in range(2):
                nc.sync.dma_start(KT[h][0][64:65, :], kaug_d.ap()[0:1, 0:L])
                for b in range(B):
                    nc.gpsimd.memset(QT[h][b][64:65, :], 1.0)
            x0_tiles[(0, 1)] = x0p.tile([128, ND * 256], F16, tag="x0",
                                        name="x01")
            nc.sync.dma_start(x0_tiles[(0, 1)][:], xh0b_d[0].ap()[:])
            nc.sync.dma_start(wv_t[:], wv_d.ap()[:])
            nc.sync.dma_start(identf_t[:], identf_d.ap()[:])

            # PE warmup: ramp the p-state and keep PE busy until the
            # first projection inputs land (a gap resets the ramp).
            wm = stripp.tile([128, 1024], F32, tag="strip",
                             name="warm")
            for _ in range(20):
                nc.tensor.matmul(wm[:, 0:128], wz_t[:], wz_t[:],
                                 start=True, stop=True)

            xg_tiles = {}   # group g (1..7) -> tile

            def load_group(g):
                t = xgp.tile([128, ND * 512], F16, tag="xg",
                             name=f"xg{g}")
                idx = g - 1 if g <= 3 else g - 2   # xhr: groups 1,2,3,5,6,7
                nc.sync.dma_start(
                    t[:], xhr_d.ap()[:, idx * 4096:(idx + 1) * 4096])
                xg_tiles[g] = t

            def load_aug_b1():
                for h in range(2):
                    nc.sync.dma_start(KT[h][1][64:65, :],
                                      kaug_d.ap()[0:1, L:2 * L])

            def x_slice(b, t0, width):
                """SBUF source tile for tokens [t0, t0+width) of batch b:
                returns (tile, per-d stride, column base)."""
                tglob = b * L + t0
                g = tglob // 512
                if g in (0, 4):   # each batch's first group: split tiles
                    half = (tglob % 512) // 256
                    return x0_tiles[(b, half)], 256, tglob % 256
                return xg_tiles[g], 512, tglob % 512

            def load_b1_first():
                for half in range(2):
                    t = x0p.tile([128, ND * 256], F16, tag="x0",
                                 name=f"x1{half}")
                    nc.sync.dma_start(t[:], (xh0a_d[1] if half == 0
                                             else xh0b_d[1]).ap()[:])
                    x0_tiles[(1, half)] = t

            # --- projection emitters ------------------------------------
            def qk_chunk_mm(b, c, dlo, dhi, pj):
                """QK projection matmuls for 256-token chunk c of batch
                b, d-chunks [dlo, dhi)."""
                t0 = 256 * c
                xt, tw, base = x_slice(b, t0, 256)
                for d in range(dlo, dhi):
                    xs = xt[:, d * tw + base:d * tw + base + 256]
                    nc.tensor.matmul(
                        pj[:, 0:256], wqk_t[:, d * 128:(d + 1) * 128],
                        xs, start=(d == 0), stop=(d == ND - 1))
                    nc.tensor.matmul(
                        pj[:, 256:512],
                        wqk_t[:, ND * 128 + d * 128:ND * 128 + (d + 1) * 128],
                        xs, start=False, stop=(d == ND - 1))

            def qk_chunk_drain(b, c, part, pj, on_act=False):
                """Drain half of chunk c: part 0 = Q, part 1 = K."""
                t0 = 256 * c
                dst = QT if part == 0 else KT
                for h in range(2):
                    src = pj[h * 64:(h + 1) * 64,
                             part * 256:part * 256 + 256]
                    d = dst[h][b][0:64, t0:t0 + 256]
                    if on_act:
                        nc.scalar.activation(d, src, AF.Identity)
                    else:
                        nc.vector.tensor_copy(d, src)

            def v_tile(b, ki):
                """V projection for token-tile ki of batch b:
                token-major via stationary-x matmul; drains into VA."""
                t0 = 128 * ki
                xt, tw, base = x_slice(b, t0, 128)
                pv = stripp.tile([128, 1024], F32, tag="strip",
                                 name="pv")
                for d in range(ND):
                    nc.tensor.matmul(
                        pv[:, 0:128],
                        xt[:, d * tw + base:d * tw + base + 128],
                        wv_t[:, d * 128:(d + 1) * 128],
                        start=(d == 0), stop=(d == ND - 1))
                nc.vector.tensor_copy(VA[b][ki][:, 1:65], pv[:, 0:64])
                nc.vector.tensor_copy(VA[b][ki][:, 66:130], pv[:, 64:128])

            # --- background queue ---------------------------------------
            bg = deque()
            emitted = set()
            by_key = {}

            def bg_add(key, pe, dve, fn, requires=()):
                it = {"key": key, "pe": pe, "dve": dve, "fn": fn,
                      "req": tuple(requires)}
                bg.append(it)
                by_key[key] = it

            def bg_run(item):
                if item["key"] in emitted:
                    return
                for r in item["req"]:
                    if r not in emitted and r in by_key:
                        bg_run(by_key[r])
                emitted.add(item["key"])
                item["fn"]()

            def pump(pe_budget, dve_budget):
                # scan past blocked items (requirements make out-of-order
                # emission safe); bounded lookahead keeps need-order bias.
                # At most one PSUM-using (PE) quantum per call: a second
                # would cycle the strip ring into a head-of-line stall.
                scanned = 0
                pe_quanta = 0
                i = 0
                while i < len(bg) and scanned < 12:
                    it = bg[i]
                    if it["key"] in emitted:
                        del bg[i]
                        continue
                    scanned += 1
                    fits = (it["pe"] <= pe_budget
                            and it["dve"] <= dve_budget
                            and (it["pe"] == 0 or pe_quanta < 1))
                    if fits:
                        del bg[i]
                        bg_run(it)
                        pe_budget -= it["pe"]
                        dve_budget -= it["dve"]
                        if it["pe"] > 0:
                            pe_quanta += 1
                    else:
                        i += 1

            def force(pred):
                # emit exactly the matching items (plus their declared
                # prerequisites), leaving unrelated queued work in place
                for it in list(bg):
                    if it["key"] not in emitted and pred(it["key"]):
                        bg_run(it)
                while bg and bg[0]["key"] in emitted:
                    bg.popleft()

            def x_req(b, t0):
                g = (b * L + t0) // 512
                if g == 0:
                    return ()
                if g == 4:
                    return (("x1a", 0),)
                return (("xg", g),)

            def queue_qk(b, c, act_drain=False):
                shared = {}
                req = x_req(b, 256 * c)

                def mk_mm(dlo, dhi):
                    def fn():
                        if "pj" not in shared:
                            shared["pj"] = stripp.tile([128, 1024], F32,
                                                       tag="strip",
                                                       name="pj")
                        qk_chunk_mm(b, c, dlo, dhi, shared["pj"])
                    return fn

                def mk_drain(part):
                    def fn():
                        qk_chunk_drain(b, c, part, shared["pj"],
                                       on_act=act_drain)
                    return fn
                for q in range(4):
                    bg_add(("qkm", b, c, q), 440, 0,
                           mk_mm(q * 2, q * 2 + 2),
                           req if q == 0 else (("qkm", b, c, q - 1),))
                bg_add(("qkd", b, c, 0), 0, 790, mk_drain(0),
                       (("qkm", b, c, 3),))
                bg_add(("qkd", b, c, 1), 0, 790, mk_drain(1),
                       (("qkm", b, c, 3),))

            def queue_v(b, ki):
                bg_add(("v", b, ki), 430, 390,
                       lambda b=b, ki=ki: v_tile(b, ki),
                       x_req(b, 128 * ki))

            def queue_dma(key, fn, requires=()):
                bg_add(key, 0, 0, fn, requires)

            # --- attention ----------------------------------------------
            def queue_store(b, blk):
                def fn(b=b, blk=blk):
                    nc.gpsimd.dma_start(
                        out_d.ap()[b, blk * 512:(blk + 1) * 512, :]
                        .rearrange("(j p) e -> p j e", p=128),
                        OST[b][blk][:].rearrange("p (j e) -> p j e", j=4))
                queue_dma(("store", b, blk), fn)

            def normalize_tile(b, h, j, dst):
                """dst: transposed [128, 65] PSUM view (col0 = denom)."""
                rec = epip.tile([128, 1], F32, tag="rec", name="rec")
                nc.vector.reciprocal(rec[:], dst[0:128, 0:1])
                blk, j_in = j // 4, j % 4
                nc.vector.tensor_scalar_mul(
                    OST[b][blk][:, j_in * 128 + h * 64:
                                j_in * 128 + h * 64 + 64],
                    dst[0:128, 1:65], rec[:])
                if h == 1:
                    if b == 1 and blk == 3:
                        # final block: per-tile stores so the tail only
                        # waits on the last 128 tokens
                        def fn(b=b, j=j, j_in=j_in, blk=blk):
                            nc.gpsimd.dma_start(
                                out_d.ap()[b, j * 128:(j + 1) * 128, :],
                                OST[b][blk][:, j_in * 128:
                                            (j_in + 1) * 128])
                        queue_dma(("store", b, blk, j), fn)
                    elif j % 4 == 3:
                        queue_store(b, blk)

            def attention_span(b, h, qlo, qw, last_span=False,
                               carry=None):
                jlo, jhi = qlo // 128, (qlo + qw) // 128
                state = {"po": None}
                prev = None
                bank_ots = {}   # bank idx -> ot tile (span-end pt chain)

                def get_po():
                    # lazy: the previous span's pt chain (carry) must hit
                    # the psO ring before this span's po allocation
                    if state["po"] is None:
                        state["po"] = pop.tile([65, 1024], F32,
                                               tag="psO", name="po")
                    return state["po"]

                def emit_av(ki, pp, q0, w):
                    # AV per 128-col tile, stop when tile j retires
                    po = get_po()
                    for j in range(max(ki, jlo), jhi):
                        cl = j * 128 - q0
                        nc.tensor.matmul(
                            po[:, j * 128 - qlo:(j + 1) * 128 - qlo],
                            VA[b][ki][:, h * 65:h * 65 + 65],
                            pp[:, cl:cl + 128],
                            start=(ki == 0 and (j - jlo) % 4 == 0),
                            stop=(ki == j))
                        if ki != j:
                            continue
                        if last_span:
                            # per-128 drain; pt from the strip ring (it
                            # interleaves with score strips without
                            # touching the po ring)
                            ot = epip.tile([65, 512], F32, tag="ot",
                                           name="ot")
                            nc.vector.tensor_copy(
                                ot[0:65, 0:128],
                                get_po()[:, j * 128 - qlo:
                                         (j + 1) * 128 - qlo])
                            pt = stripp.tile([128, 1024], F32,
                                             tag="strip", name="pt")
                            nc.tensor.transpose(pt[0:128, 0:65],
                                                ot[0:65, 0:128],
                                                identf_t[0:65, 0:65])
                            normalize_tile(b, h, j, pt[0:128, 0:65])
                        elif j == min(((j - jlo) // 4) * 4 + 3 + jlo,
                                      jhi - 1):
                            # bank complete: drain to SBUF now, pt chain
                            # at span end (po ring: pts must follow the
                            # last po access)
                            bk = (j - jlo) // 4
                            jb = bk * 4 + jlo
                            nb = j - jb + 1
                            ot = epip.tile([65, 512], F32, tag="ot",
                                           name="ot")
                            nc.vector.tensor_copy(
                                ot[0:65, 0:nb * 128],
                                get_po()[:, jb * 128 - qlo:
                                         (jb + nb) * 128 - qlo])
                            bank_ots[bk] = (ot, jb, nb)

                sc = {}

                def emit_score(ki):
                    q0 = max(qlo, ki * 128)
                    w = qlo + qw - q0
                    strip = stripp.tile([128, 1024], F32, tag="strip",
                                        name="strip")
                    for off in range(0, w, 512):
                        ln = min(512, w - off)
                        nc.tensor.matmul(
                            strip[:, off:off + ln],
                            KT[h][b][:, ki * 128:ki * 128 + 128],
                            QT[h][b][:, q0 + off:q0 + off + ln],
                            start=True,
                            stop=True)
                    sc[ki] = (strip, q0, w)

                emit_score(0)
                for ki in range(jhi):
                    # prefetch next score so it runs ahead of this
                    # iteration's AV/pump on the in-order PE queue
                    if ki + 1 < jhi:
                        emit_score(ki + 1)
                    strip, q0, w = sc.pop(ki)
                    diag = (q0 == ki * 128)
                    nc.scalar.activation(strip[:, 0:w], strip[:, 0:w],
                                         AF.Tanh, scale=0.125)
                    pp = ppp.tile([128, 1024], BF16, tag="pp", name="pp")
                    nc.scalar.activation(pp[:, 0:w], strip[:, 0:w],
                                         AF.Exp, bias=n30_t[:],
                                         scale=TAU)
                    if diag:
                        # exact-zero the below-diagonal ghosts (the tanh
                        # saturation trick leaves them at e^-60, which
                        # contaminates rows whose live P's are comparable)
                        nc.vector.tensor_mul(pp[:, 0:128], pp[:, 0:128],
                                             trilb_t[:])
                    if carry is not None:
                        carry()
                        carry = None
                    if prev is not None:
                        if not ("v", b, prev[0]) in emitted:
                            force(lambda k, kk=prev[0]:
                                  k == ("v", b, kk))
                        emit_av(*prev)
                    act_ns = 1.67 * w + 330
                    pe_ns = 0.43 * w + (60 if diag else 0) + \
                        0.43 * 128 * max(0, jhi - max(ki - 1, jlo)) + 100
                    pump(max(0.0, act_ns - pe_ns - 100),
                         max(0.0, act_ns - 550))
                    prev = (ki, pp, q0, w)
                if prev is not None:
                    if not ("v", b, prev[0]) in emitted:
                        force(lambda k, kk=prev[0]: k == ("v", b, kk))
                    emit_av(*prev)
                # span-end pt chain (non-last spans): all po accesses
                # are emitted, so psO-ring pt tiles may cycle the slot;
                # returned as a closure so the NEXT span's first
                # iteration can overlap it
                def finish():
                    for bk in sorted(bank_ots):
                        ot, jb, nb = bank_ots[bk]
                        for jj in range(nb):
                            pt = pop.tile([128, 65], F32, tag="psO",
                                          name="pt")
                            nc.tensor.transpose(
                                pt[:], ot[0:65, jj * 128:(jj + 1) * 128],
                                identf_t[0:65, 0:65])
                            normalize_tile(b, h, jb + jj, pt[:])
                return finish

            # --- orchestration ------------------------------------------
            # Spans alternate heads: h1 re-uses h0's projections, so the
            # ACT work per projection deadline doubles and background
            # projection quanta fit inside the span slack.
            # prologue: QK chunk 0 + V tiles 0-1 of b0 (drains split
            # ACT/DVE to shorten the startup chain)
            pj0 = stripp.tile([128, 1024], F32, tag="strip",
                              name="pj0")
            xt0, tw0, _ = x_slice(0, 0, 256)
            for d in range(ND):
                nc.tensor.matmul(pj0[:, 0:256],
                                 wqk_t[:, d * 128:(d + 1) * 128],
                                 xt0[:, d * tw0:d * tw0 + 256],
                                 start=(d == 0), stop=(d == ND - 1))
            qk_chunk_drain(0, 0, 0, pj0, on_act=True)
            for d in range(ND):
                nc.tensor.matmul(
                    pj0[:, 256:512],
                    wqk_t[:, ND * 128 + d * 128:ND * 128 + (d + 1) * 128],
                    xt0[:, d * tw0:d * tw0 + 256],
                    start=False, stop=(d == ND - 1))
            qk_chunk_drain(0, 0, 1, pj0, on_act=False)
            for ki in (0, 1):
                queue_v(0, ki)

            # chunk 1 matmuls inline during the ACT-idle startup;
            # its drains go to the background queue
            pj1 = stripp.tile([128, 1024], F32, tag="strip",
                              name="pj1")
            qk_chunk_mm(0, 1, 0, ND, pj1)
            for part in range(2):
                bg_add(("qkd", 0, 1, part), 0, 790,
                       lambda p=part: qk_chunk_drain(0, 1, p, pj1))
            for q in range(4):
                emitted.add(("qkm", 0, 1, q))

            # b0 background work, queued in need order
            for g in (1, 2, 3):
                queue_dma(("xg", g), lambda g=g: load_group(g))
            for c in (2, 3):
                queue_qk(0, c)
            for ki in (2, 3, 4, 5):
                queue_v(0, ki)
            for c in (4, 5):
                queue_qk(0, c)
            for ki in (6, 7, 8, 9):
                queue_v(0, ki)
            for c in (6, 7):
                queue_qk(0, c)
            for ki in range(10, NJ):
                queue_v(0, ki)

            def need_qk(b, cs):
                force(lambda k: k[0] in ("qkm", "qkd")
                      and k[1] == b and k[2] in cs)

            # batch 0, head-alternating spans
            cr = attention_span(0, 0, 0, 256)
            cr = attention_span(0, 1, 0, 256, carry=cr)
            need_qk(0, (1,))
            cr = attention_span(0, 0, 256, 256, carry=cr)
            cr = attention_span(0, 1, 256, 256, carry=cr)
            need_qk(0, (2, 3))
            cr = attention_span(0, 0, 512, 512, carry=cr)
            need_qk(0, (4, 5))
            cr = attention_span(0, 1, 512, 512, carry=cr)
            # queue b1 inputs + b1 projection work mid-flight
            queue_dma(("x1a", 0), load_b1_first)
            queue_dma(("aug1", 0), load_aug_b1)
            for g in (5, 6, 7):
                queue_dma(("xg", g), lambda g=g: load_group(g))
            for c in (0, 1, 2, 3):
                queue_qk(1, c)
            for ki in (0, 1, 2, 3):
                queue_v(1, ki)
            for c in (4, 5, 6, 7):
                queue_qk(1, c)
            for ki in range(4, NJ):
                queue_v(1, ki)
            need_qk(0, (6, 7))
            cr = attention_span(0, 0, 1024, 1024, carry=cr)
            cr = attention_span(0, 1, 1024, 1024, carry=cr)

            # batch 1
            force(lambda k: k[0] in ("x1a", "aug1"))
            need_qk(1, (0, 1, 2, 3))
            cr = attention_span(1, 0, 0, 1024, carry=cr)
            cr = attention_span(1, 1, 0, 1024, carry=cr)
            need_qk(1, (4, 5, 6, 7))
            cr = attention_span(1, 0, 1024, 1024, carry=cr)
            cr = attention_span(1, 1, 1024, 1024, last_span=True,
                                carry=cr)
            cr()
            force(lambda k: True)

    nc.compile()
    return nc


def _get_program():
    if "nc" not in _CACHE:
        _CACHE["nc"] = _build_program()
    return _CACHE["nc"]


def _prep_inputs(input, attention_mask, W_Q, W_K, W_V):
    f16 = np.float16

    x = np.asarray(input, dtype=np.float32).reshape(T, D)
    xT = np.ascontiguousarray(x.T).astype(f16)             # [D, T]
    # [d, p, g, tloc]
    xr = xT.reshape(ND, 128, B * 4, 512).transpose(1, 2, 0, 3)
    # xr: [p, g, d, tloc]
    xh = {}
    for b in range(B):
        g0 = xr[:, b * 4]                                    # [p, d, 512]
        xh[f"xh0a{b}"] = np.ascontiguousarray(
            g0[:, :, 0:256].reshape(128, ND * 256))
        xh[f"xh0b{b}"] = np.ascontiguousarray(
            g0[:, :, 256:512].reshape(128, ND * 256))
    rest = np.concatenate([xr[:, g] for g in (1, 2, 3, 5, 6, 7)],
                          axis=1)                            # [p, 6*d, 512]
    xhr = np.ascontiguousarray(rest.reshape(128, 6 * ND * 512))

    import ml_dtypes
    mask = np.asarray(attention_mask).astype(np.float32).reshape(1, T)
    kaug = ((mask - 1.0) * NEG_BIG).astype(f16)
    qi = np.arange(128)
    trilb = np.where(qi[None, :] >= qi[:, None], 1.0,
                     0.0).astype(ml_dtypes.bfloat16)   # keep[k,q]: q >= k
    identf = np.eye(128, dtype=np.float32)

    common = {
        **xh, "xhr": xhr, "kaug": kaug, "trilb": trilb,
        "identf": identf,
    }

    def pack_w(Wm, sl):
        wc = np.asarray(Wm, np.float32)[sl, :].astype(f16)  # [128e, D]
        return wc.reshape(128, ND, 128).transpose(2, 1, 0)   # [p, d, e]

    in_maps = []
    for c in range(N_CORES):
        sl = slice(c * E, (c + 1) * E)
        wq = pack_w(W_Q, sl)
        wk = pack_w(W_K, sl)
        wv = pack_w(W_V, sl)
        in_maps.append({
            **common,
            "wqk": np.ascontiguousarray(
                np.concatenate([wq, wk], axis=1).reshape(128, 2 * ND * 128)),
            "wv": np.ascontiguousarray(wv.reshape(128, ND * 128)),
        })
    return in_maps


def kernel(input, attention_mask, W_Q, W_K, W_V):
    from concourse.bass_utils import run_bass_kernel_spmd

    nc = _get_program()
    in_maps = _prep_inputs(input, attention_mask, W_Q, W_K, W_V)
    res = run_bass_kernel_spmd(nc, in_maps, list(range(N_CORES)))
    return np.concatenate([res.results[c]["out"] for c in range(N_CORES)],
                          axis=2)


# revision 47
# speedup vs baseline: 1.1888x; 1.0184x over previous
"""Trainium2 Bass kernel for sparse (causal, tanh-clamped) attention.

Problem: B=2, L=2048, D=1024, H=16 heads x 64 dim; S = QK^T/8;
S = 30*tanh(S); causal + attention_mask; softmax; out = attn @ V.

Sharding: 2 heads per core across 8 cores (tensor-parallel on heads).

v2 design (ACT is the bottleneck engine: 2 passes tanh+exp over all
causal score columns ~= 116us + per-instruction overheads ~= 150us):
 - bf16 inputs/weights/QKV/probabilities (matmul cost is per moving row
   regardless of dtype>=bf16; halves DMA bytes and removes the fp32r
   <256-row 4x penalty).
 - S^T[k, q] layout throughout; no P transpose (P^T feeds AV directly).
 - attention_mask folded into the score matmul via 65th contraction row.
 - causal diagonal-block mask applied INSIDE the score accumulation as
   one extra matmul (maskT stationary, identity moving): no DVE/ACT cost.
 - bounded logits: P = exp(30*tanh(s)-30), no running max; denominator
   via a ones-column in V (column 0 of the augmented V tile).
 - V is projected token-major directly (stationary=x chunk, moving=W_V
   chunk) -- no V transposes, no separate vaug copies.
 - AV accumulation per 128-column tile with stop at ki==j: epilogues
   drain per-512 bank as soon as its last k-tile lands (spread through
   the run; tiny tail).
 - software-pipelined emission: per k-tile iteration emit score(ki),
   tanh/exp(ki), then AV(ki-1) and budget-limited background quanta
   (projections/V/stores) so the in-order PE queue never starves ACT.
"""

import sys

if "/opt/trn_rl_repo" not in sys.path:
    sys.path.insert(0, "/opt/trn_rl_repo")

import numpy as np

B = 2
L = 2048
D = 1024
H = 16
DH = 64
N_CORES = 8
T = B * L            # 4096 tokens
E = 128              # per-core output features (2 heads)
ND = D // 128        # 8 contraction chunks
NEG_BIG = 6.0e4   # fp16-safe; still saturates tanh
TAU = 30.0

_CACHE = {}


def _build_program():
    import concourse.bacc as bacc
    import concourse.tile as tile
    from concourse import mybir
    from collections import deque

    F32 = mybir.dt.float32
    F16 = mybir.dt.float16
    BF16 = mybir.dt.bfloat16
    AF = mybir.ActivationFunctionType

    nc = bacc.Bacc("TRN2", target_bir_lowering=False, debug=False,
                   num_devices=N_CORES)

    # --- DRAM tensors -----------------------------------------------------
    # x host-packed: group0 of each batch split in two 256-token pieces
    # laid out [p, (d, t)], remaining groups [p, (g, d, t512)].
    xh0a_d = [nc.dram_tensor(f"xh0a{b}", [128, ND * 256], F16,
                             kind="ExternalInput") for b in range(B)]
    xh0b_d = [nc.dram_tensor(f"xh0b{b}", [128, ND * 256], F16,
                             kind="ExternalInput") for b in range(B)]
    xhr_d = nc.dram_tensor("xhr", [128, 6 * ND * 512], F16,
                           kind="ExternalInput")
    wqk_d = nc.dram_tensor("wqk", [128, 2 * ND * 128], F16,
                           kind="ExternalInput")
    wv_d = nc.dram_tensor("wv", [128, ND * 128], F16,
                          kind="ExternalInput")
    kaug_d = nc.dram_tensor("kaug", [1, T], F16, kind="ExternalInput")
    trilb_d = nc.dram_tensor("trilb", [128, 128], BF16,
                             kind="ExternalInput")
    identf_d = nc.dram_tensor("identf", [128, 128], F32,
                              kind="ExternalInput")
    out_d = nc.dram_tensor("out", [B, L, E], F32, kind="ExternalOutput")

    NJ = L // 128     # 16 column tiles per sequence

    with tile.TileContext(nc) as tc:
        with (
            tc.tile_pool(name="const", bufs=1) as constp,
            tc.tile_pool(name="weights", bufs=1) as wp,
            tc.tile_pool(name="qk", bufs=1) as qkp,
            tc.tile_pool(name="va", bufs=1) as vap,
            tc.tile_pool(name="x0", bufs=4) as x0p,
            tc.tile_pool(name="xg", bufs=6) as xgp,
            tc.tile_pool(name="prob", bufs=6) as ppp,
            tc.tile_pool(name="epi", bufs=6) as epip,
            tc.tile_pool(name="ostage", bufs=1) as ostagep,
            tc.tile_pool(name="strip", bufs=3, space="PSUM") as stripp,
            tc.tile_pool(name="psO", bufs=1, space="PSUM") as pop,
        ):
            # --- constants ----------------------------------------------
            trilb_t = constp.tile([128, 128], BF16, tag="trilb")
            identf_t = constp.tile([128, 128], F32, tag="identf")
            n30_t = constp.tile([128, 1], F32, tag="n30")
            nc.gpsimd.memset(n30_t[:], -TAU)
            wz_t = constp.tile([128, 128], F16, tag="wzero")
            nc.gpsimd.memset(wz_t[:], 0.0)

            wqk_t = wp.tile([128, 2 * ND * 128], F16, tag="wqk",
                            name="wqk")
            wv_t = wp.tile([128, ND * 128], F16, tag="wv", name="wv")

            QT = [[qkp.tile([65, L], F16, tag=f"qt{h}{b}",
                            name=f"qt{h}{b}")
                   for b in range(B)] for h in range(2)]
            KT = [[qkp.tile([65, L], F16, tag=f"kt{h}{b}",
                            name=f"kt{h}{b}")
                   for b in range(B)] for h in range(2)]
            # va[b][ki]: [128 tokens, 131]: col0=ones, 1:65=h0 feats,
            # col65=ones, 66:130=h1 feats
            VA = [[vap.tile([128, 131], BF16, tag=f"va{b}_{k}",
                            name=f"va{b}_{k}") for k in range(NJ)]
                  for b in range(B)]
            OST = [[ostagep.tile([128, 512], F32, tag=f"os{b}_{k}",
                                 name=f"os{b}_{k}") for k in range(4)]
                   for b in range(B)]

            # ones columns of VA (written once; Pool is idle)
            for b in range(B):
                for k in range(NJ):
                    nc.gpsimd.memset(VA[b][k][:, 0:1], 1.0)
                    nc.gpsimd.memset(VA[b][k][:, 65:66], 1.0)

            # --- initial DMAs. The startup chain is HWDGE-bound
            # (~625ns per dma_start on a serial device), so the minimum
            # number of transfers gates the first projection.
            x0_tiles = {}   # (b, half) -> tile
            x0_tiles[(0, 0)] = x0p.tile([128, ND * 256], F16, tag="x0",
                                        name="x00")
            nc.sync.dma_start(x0_tiles[(0, 0)][:], xh0a_d[0].ap()[:])
            nc.sync.dma_start(wqk_t[:, 0:1024], wqk_d.ap()[:, 0:1024])
            nc.sync.dma_start(wqk_t[:, 1024:2048],
                              wqk_d.ap()[:, 1024:2048])
            for h in range(2):
                nc.sync.dma_start(KT[h][0][64:65, :], kaug_d.ap()[0:1, 0:L])
                for b in range(B):
                    nc.gpsimd.memset(QT[h][b][64:65, :], 1.0)
            x0_tiles[(0, 1)] = x0p.tile([128, ND * 256], F16, tag="x0",
                                        name="x01")
            nc.sync.dma_start(x0_tiles[(0, 1)][:], xh0b_d[0].ap()[:])
            nc.sync.dma_start(trilb_t[:], trilb_d.ap()[:])
            nc.sync.dma_start(wv_t[:], wv_d.ap()[:])
            nc.sync.dma_start(identf_t[:], identf_d.ap()[:])

            # PE warmup: ramp the p-state and keep PE busy until the
            # first projection inputs land (a gap resets the ramp).
            wm = stripp.tile([128, 1024], F32, tag="strip",
                             name="warm")
            for _ in range(20):
                nc.tensor.matmul(wm[:, 0:128], wz_t[:], wz_t[:],
                                 start=True, stop=True)

            xg_tiles = {}   # group g (1..7) -> tile

            def load_group(g):
                t = xgp.tile([128, ND * 512], F16, tag="xg",
                             name=f"xg{g}")
                idx = g - 1 if g <= 3 else g - 2   # xhr: groups 1,2,3,5,6,7
                nc.sync.dma_start(
                    t[:], xhr_d.ap()[:, idx * 4096:(idx + 1) * 4096])
                xg_tiles[g] = t

            def load_aug_b1():
                for h in range(2):
                    nc.sync.dma_start(KT[h][1][64:65, :],
                                      kaug_d.ap()[0:1, L:2 * L])

            def x_slice(b, t0, width):
                """SBUF source tile for tokens [t0, t0+width) of batch b:
                returns (tile, per-d stride, column base)."""
                tglob = b * L + t0
                g = tglob // 512
                if g in (0, 4):   # each batch's first group: split tiles
                    half = (tglob % 512) // 256
                    return x0_tiles[(b, half)], 256, tglob % 256
                return xg_tiles[g], 512, tglob % 512

            def load_b1_first():
                for half in range(2):
                    t = x0p.tile([128, ND * 256], F16, tag="x0",
                                 name=f"x1{half}")
                    nc.sync.dma_start(t[:], (xh0a_d[1] if half == 0
                                             else xh0b_d[1]).ap()[:])
                    x0_tiles[(1, half)] = t

            # --- projection emitters ------------------------------------
            def qk_chunk_mm(b, c, dlo, dhi, pj):
                """QK projection matmuls for 256-token chunk c of batch
                b, d-chunks [dlo, dhi)."""
                t0 = 256 * c
                xt, tw, base = x_slice(b, t0, 256)
                for d in range(dlo, dhi):
                    xs = xt[:, d * tw + base:d * tw + base + 256]
                    nc.tensor.matmul(
                        pj[:, 0:256], wqk_t[:, d * 128:(d + 1) * 128],
                        xs, start=(d == 0), stop=(d == ND - 1))
                    nc.tensor.matmul(
                        pj[:, 256:512],
                        wqk_t[:, ND * 128 + d * 128:ND * 128 + (d + 1) * 128],
                        xs, start=False, stop=(d == ND - 1))

            def qk_chunk_drain(b, c, part, pj, on_act=False, hs=(0, 1)):
                """Drain chunk c: part 0 = Q, part 1 = K; heads hs."""
                t0 = 256 * c
                dst = QT if part == 0 else KT
                for h in hs:
                    src = pj[h * 64:(h + 1) * 64,
                             part * 256:part * 256 + 256]
                    d = dst[h][b][0:64, t0:t0 + 256]
                    if on_act:
                        nc.scalar.activation(d, src, AF.Identity)
                    else:
                        nc.vector.tensor_copy(d, src)

            def v_tile(b, ki):
                """V projection for token-tile ki of batch b:
                token-major via stationary-x matmul; drains into VA."""
                t0 = 128 * ki
                xt, tw, base = x_slice(b, t0, 128)
                pv = stripp.tile([128, 1024], F32, tag="strip",
                                 name="pv")
                for d in range(ND):
                    nc.tensor.matmul(
                        pv[:, 0:128],
                        xt[:, d * tw + base:d * tw + base + 128],
                        wv_t[:, d * 128:(d + 1) * 128],
                        start=(d == 0), stop=(d == ND - 1))
                nc.vector.tensor_copy(VA[b][ki][:, 1:65], pv[:, 0:64])
                nc.vector.tensor_copy(VA[b][ki][:, 66:130], pv[:, 64:128])

            # --- background queue ---------------------------------------
            bg = deque()
            emitted = set()
            by_key = {}

            def bg_add(key, pe, dve, fn, requires=()):
                it = {"key": key, "pe": pe, "dve": dve, "fn": fn,
                      "req": tuple(requires)}
                bg.append(it)
                by_key[key] = it

            def bg_run(item):
                if item["key"] in emitted:
                    return
                for r in item["req"]:
                    if r not in emitted and r in by_key:
                        bg_run(by_key[r])
                emitted.add(item["key"])
                item["fn"]()

            def pump(pe_budget, dve_budget):
                # scan past blocked items (requirements make out-of-order
                # emission safe); bounded lookahead keeps need-order bias.
                # At most one PSUM-using (PE) quantum per call: a second
                # would cycle the strip ring into a head-of-line stall.
                scanned = 0
                pe_quanta = 0
                i = 0
                while i < len(bg) and scanned < 12:
                    it = bg[i]
                    if it["key"] in emitted:
                        del bg[i]
                        continue
                    scanned += 1
                    fits = (it["pe"] <= pe_budget
                            and it["dve"] <= dve_budget
                            and (it["pe"] == 0 or pe_quanta < 1))
                    if fits:
                        del bg[i]
                        bg_run(it)
                        pe_budget -= it["pe"]
                        dve_budget -= it["dve"]
                        if it["pe"] > 0:
                            pe_quanta += 1
                    else:
                        i += 1

            def force(pred):
                # emit exactly the matching items (plus their declared
                # prerequisites), leaving unrelated queued work in place
                for it in list(bg):
                    if it["key"] not in emitted and pred(it["key"]):
                        bg_run(it)
                while bg and bg[0]["key"] in emitted:
                    bg.popleft()

            def x_req(b, t0):
                g = (b * L + t0) // 512
                if g == 0:
                    return ()
                if g == 4:
                    return (("x1a", 0),)
                return (("xg", g),)

            def queue_qk(b, c, act_drain=False):
                shared = {}
                req = x_req(b, 256 * c)

                def mk_mm(dlo, dhi):
                    def fn():
                        if "pj" not in shared:
                            shared["pj"] = stripp.tile([128, 1024], F32,
                                                       tag="strip",
                                                       name="pj")
                        qk_chunk_mm(b, c, dlo, dhi, shared["pj"])
                    return fn

                def mk_drain(part):
                    def fn():
                        qk_chunk_drain(b, c, part, shared["pj"],
                                       on_act=act_drain)
                    return fn
                for q in range(4):
                    bg_add(("qkm", b, c, q), 440, 0,
                           mk_mm(q * 2, q * 2 + 2),
                           req if q == 0 else (("qkm", b, c, q - 1),))
                bg_add(("qkd", b, c, 0), 0, 790, mk_drain(0),
                       (("qkm", b, c, 3),))
                bg_add(("qkd", b, c, 1), 0, 790, mk_drain(1),
                       (("qkm", b, c, 3),))

            def queue_v(b, ki):
                bg_add(("v", b, ki), 430, 390,
                       lambda b=b, ki=ki: v_tile(b, ki),
                       x_req(b, 128 * ki))

            def queue_dma(key, fn, requires=()):
                bg_add(key, 0, 0, fn, requires)

            # --- attention ----------------------------------------------
            def queue_store(b, blk):
                def fn(b=b, blk=blk):
                    nc.gpsimd.dma_start(
                        out_d.ap()[b, blk * 512:(blk + 1) * 512, :]
                        .rearrange("(j p) e -> p j e", p=128),
                        OST[b][blk][:].rearrange("p (j e) -> p j e", j=4))
                queue_dma(("store", b, blk), fn)

            def normalize_tile(b, h, j, dst):
                """dst: transposed [128, 65] PSUM view (col0 = denom)."""
                rec = epip.tile([128, 1], F32, tag="rec", name="rec")
                nc.vector.reciprocal(rec[:], dst[0:128, 0:1])
                blk, j_in = j // 4, j % 4
                nc.vector.tensor_scalar_mul(
                    OST[b][blk][:, j_in * 128 + h * 64:
                                j_in * 128 + h * 64 + 64],
                    dst[0:128, 1:65], rec[:])
                if h == 1:
                    if b == 1 and blk == 3:
                        # final block: per-tile stores on the sync queue
                        # (HWDGE; the SWDGE gen on Pool would serialize
                        # the last three stores at ~1us each)
                        def fn(b=b, j=j, j_in=j_in, blk=blk):
                            nc.sync.dma_start(
                                out_d.ap()[b, j * 128:(j + 1) * 128, :],
                                OST[b][blk][:, j_in * 128:
                                            (j_in + 1) * 128])
                        queue_dma(("store", b, blk, j), fn)
                    elif j % 4 == 3:
                        queue_store(b, blk)

            def attention_span(b, h, qlo, qw, last_span=False,
                               carry=None, trickle=None):
                jlo, jhi = qlo // 128, (qlo + qw) // 128
                state = {"po": None}
                prev = None
                bank_ots = {}   # bank idx -> ot tile (span-end pt chain)

                def get_po():
                    # lazy: the previous span's pt chain (carry) must hit
                    # the psO ring before this span's po allocation
                    if state["po"] is None:
                        state["po"] = pop.tile([65, 1024], F32,
                                               tag="psO", name="po")
                    return state["po"]

                def emit_av(ki, pp, q0, w):
                    # AV per 128-col tile, stop when tile j retires
                    po = get_po()
                    for j in range(max(ki, jlo), jhi):
                        cl = j * 128 - q0
                        nc.tensor.matmul(
                            po[:, j * 128 - qlo:(j + 1) * 128 - qlo],
                            VA[b][ki][:, h * 65:h * 65 + 65],
                            pp[:, cl:cl + 128],
                            start=(ki == 0 and (j - jlo) % 4 == 0),
                            stop=(ki == j))
                        if ki != j:
                            continue
                        if last_span:
                            # per-128 drain; pt from the strip ring (it
                            # interleaves with score strips without
                            # touching the po ring)
                            ot = epip.tile([65, 512], F32, tag="ot",
                                           name="ot")
                            nc.vector.tensor_copy(
                                ot[0:65, 0:128],
                                get_po()[:, j * 128 - qlo:
                                         (j + 1) * 128 - qlo])
                            pt = stripp.tile([128, 1024], F32,
                                             tag="strip", name="pt")
                            nc.tensor.transpose(pt[0:128, 0:65],
                                                ot[0:65, 0:128],
                                                identf_t[0:65, 0:65])
                            normalize_tile(b, h, j, pt[0:128, 0:65])
                        elif j == min(((j - jlo) // 4) * 4 + 3 + jlo,
                                      jhi - 1):
                            # bank complete: drain to SBUF now, pt chain
                            # at span end (po ring: pts must follow the
                            # last po access)
                            bk = (j - jlo) // 4
                            jb = bk * 4 + jlo
                            nb = j - jb + 1
                            ot = epip.tile([65, 512], F32, tag="ot",
                                           name="ot")
                            nc.vector.tensor_copy(
                                ot[0:65, 0:nb * 128],
                                get_po()[:, jb * 128 - qlo:
                                         (jb + nb) * 128 - qlo])
                            bank_ots[bk] = (ot, jb, nb)

                sc = {}

                def emit_score(ki):
                    q0 = max(qlo, ki * 128)
                    w = qlo + qw - q0
                    strip = stripp.tile([128, 1024], F32, tag="strip",
                                        name="strip")
                    for off in range(0, w, 512):
                        ln = min(512, w - off)
                        nc.tensor.matmul(
                            strip[:, off:off + ln],
                            KT[h][b][:, ki * 128:ki * 128 + 128],
                            QT[h][b][:, q0 + off:q0 + off + ln],
                            start=True,
                            stop=True)
                    sc[ki] = (strip, q0, w)

                emit_score(0)
                for ki in range(jhi):
                    # prefetch next score so it runs ahead of this
                    # iteration's AV/pump on the in-order PE queue
                    if ki + 1 < jhi:
                        emit_score(ki + 1)
                    strip, q0, w = sc.pop(ki)
                    diag = (q0 == ki * 128)
                    nc.scalar.activation(strip[:, 0:w], strip[:, 0:w],
                                         AF.Tanh, scale=0.125)
                    pp = ppp.tile([128, 1024], BF16, tag="pp", name="pp")
                    nc.scalar.activation(pp[:, 0:w], strip[:, 0:w],
                                         AF.Exp, bias=n30_t[:],
                                         scale=TAU)
                    if diag:
                        # exact-zero the below-diagonal ghosts (the tanh
                        # saturation trick leaves them at e^-60, which
                        # contaminates rows whose live P's are comparable)
                        nc.vector.tensor_mul(pp[:, 0:128], pp[:, 0:128],
                                             trilb_t[:])
                    if carry is not None:
                        carry()
                        carry = None
                    if prev is not None:
                        if not ("v", b, prev[0]) in emitted:
                            force(lambda k, kk=prev[0]:
                                  k == ("v", b, kk))
                        emit_av(*prev)
                    act_ns = 1.67 * w + 330
                    pe_ns = 0.43 * w + (60 if diag else 0) + \
                        0.43 * 128 * max(0, jhi - max(ki - 1, jlo)) + 100
                    pump(max(0.0, act_ns - pe_ns - 100),
                         max(0.0, act_ns - 550))
                    if trickle:
                        k = trickle.popleft()
                        if k in by_key and k not in emitted:
                            bg_run(by_key[k])
                    prev = (ki, pp, q0, w)
                if prev is not None:
                    if not ("v", b, prev[0]) in emitted:
                        force(lambda k, kk=prev[0]: k == ("v", b, kk))
                    emit_av(*prev)
                # span-end pt chain (non-last spans): all po accesses
                # are emitted, so psO-ring pt tiles may cycle the slot;
                # returned as a closure so the NEXT span's first
                # iteration can overlap it
                def finish():
                    for bk in sorted(bank_ots):
                        ot, jb, nb = bank_ots[bk]
                        for jj in range(nb):
                            pt = pop.tile([128, 65], F32, tag="psO",
                                          name="pt")
                            nc.tensor.transpose(
                                pt[:], ot[0:65, jj * 128:(jj + 1) * 128],
                                identf_t[0:65, 0:65])
                            normalize_tile(b, h, jb + jj, pt[:])
                return finish

            # --- orchestration ------------------------------------------
            # Spans alternate heads: h1 re-uses h0's projections, so the
            # ACT work per projection deadline doubles and background
            # projection quanta fit inside the span slack.
            # prologue: QK chunk 0 + V tiles 0-1 of b0 (drains split
            # ACT/DVE to shorten the startup chain)
            pj0 = stripp.tile([128, 1024], F32, tag="strip",
                              name="pj0")
            qk_chunk_mm(0, 0, 0, ND, pj0)
            qk_chunk_drain(0, 0, 0, pj0, on_act=True)
            qk_chunk_drain(0, 0, 1, pj0, on_act=False)
            for ki in (0, 1):
                queue_v(0, ki)

            # chunk 1 matmuls inline during the ACT-idle startup;
            # its drains go to the background queue
            pj1 = stripp.tile([128, 1024], F32, tag="strip",
                              name="pj1")
            qk_chunk_mm(0, 1, 0, ND, pj1)
            for part in range(2):
                bg_add(("qkd", 0, 1, part), 0, 790,
                       lambda p=part: qk_chunk_drain(0, 1, p, pj1))
            for q in range(4):
                emitted.add(("qkm", 0, 1, q))

            for g in (1, 2, 3):
                queue_dma(("xg", g), lambda g=g: load_group(g))
            for c in (2, 3):
                queue_qk(0, c)
            for ki in (2, 3, 4, 5):
                queue_v(0, ki)
            for c in (4, 5):
                queue_qk(0, c)
            for ki in (6, 7, 8, 9):
                queue_v(0, ki)
            for c in (6, 7):
                queue_qk(0, c)
            for ki in range(10, NJ):
                queue_v(0, ki)

            def need_qk(b, cs):
                force(lambda k: k[0] in ("qkm", "qkd")
                      and k[1] == b and k[2] in cs)

            def tk(*keys):
                return deque(keys)

            def chunk_keys(b, *cs):
                out = []
                for c in cs:
                    out += [("qkm", b, c, q) for q in range(4)]
                    out += [("qkd", b, c, p, hh)
                            for p in range(2) for hh in range(2)]
                return deque(out)

            # batch 0, head-alternating spans; trickle upcoming chunk
            # quanta through the narrow spans (the budget pump can't
            # fit them there)
            cr = attention_span(0, 0, 0, 256)
            cr = attention_span(0, 1, 0, 256, carry=cr)
            force(lambda k: k[0] == "qkd" and k[1] == 0 and k[2] == 1)
            cr = attention_span(0, 0, 256, 256, carry=cr)
            cr = attention_span(0, 1, 256, 256, carry=cr)
            need_qk(0, (2, 3))
            cr = attention_span(0, 0, 512, 512, carry=cr)
            need_qk(0, (4, 5))
            cr = attention_span(0, 1, 512, 512, carry=cr)
            # queue b1 inputs + b1 projection work mid-flight
            queue_dma(("x1a", 0), load_b1_first)
            queue_dma(("aug1", 0), load_aug_b1)
            for g in (5, 6, 7):
                queue_dma(("xg", g), lambda g=g: load_group(g))
            for c in (0, 1, 2, 3):
                queue_qk(1, c)
            for ki in (0, 1, 2, 3):
                queue_v(1, ki)
            for c in (4, 5, 6, 7):
                queue_qk(1, c)
            for ki in range(4, NJ):
                queue_v(1, ki)
            need_qk(0, (6, 7))
            cr = attention_span(0, 0, 1024, 1024, carry=cr)
            cr = attention_span(0, 1, 1024, 1024, carry=cr)

            # batch 1
            force(lambda k: k[0] in ("x1a", "aug1"))
            need_qk(1, (0, 1, 2, 3))
            cr = attention_span(1, 0, 0, 1024, carry=cr)
            cr = attention_span(1, 1, 0, 1024, carry=cr)
            need_qk(1, (4, 5, 6, 7))
            cr = attention_span(1, 0, 1024, 1024, carry=cr)
            cr = attention_span(1, 1, 1024, 1024, last_span=True,
                                carry=cr)
            cr()
            force(lambda k: True)

    nc.compile()
    return nc


def _get_program():
    if "nc" not in _CACHE:
        _CACHE["nc"] = _build_program()
    return _CACHE["nc"]


def _prep_inputs(input, attention_mask, W_Q, W_K, W_V):
    f16 = np.float16

    x = np.asarray(input, dtype=np.float32).reshape(T, D)
    xT = np.ascontiguousarray(x.T).astype(f16)             # [D, T]
    # [d, p, g, tloc]
    xr = xT.reshape(ND, 128, B * 4, 512).transpose(1, 2, 0, 3)
    # xr: [p, g, d, tloc]
    xh = {}
    for b in range(B):
        g0 = xr[:, b * 4]                                    # [p, d, 512]
        xh[f"xh0a{b}"] = np.ascontiguousarray(
            g0[:, :, 0:256].reshape(128, ND * 256))
        xh[f"xh0b{b}"] = np.ascontiguousarray(
            g0[:, :, 256:512].reshape(128, ND * 256))
    rest = np.concatenate([xr[:, g] for g in (1, 2, 3, 5, 6, 7)],
                          axis=1)                            # [p, 6*d, 512]
    xhr = np.ascontiguousarray(rest.reshape(128, 6 * ND * 512))

    import ml_dtypes
    mask = np.asarray(attention_mask).astype(np.float32).reshape(1, T)
    kaug = ((mask - 1.0) * NEG_BIG).astype(f16)
    qi = np.arange(128)
    trilb = np.where(qi[None, :] >= qi[:, None], 1.0,
                     0.0).astype(ml_dtypes.bfloat16)   # keep[k,q]: q >= k
    identf = np.eye(128, dtype=np.float32)

    common = {
        **xh, "xhr": xhr, "kaug": kaug, "trilb": trilb,
        "identf": identf,
    }

    def pack_w(Wm, sl):
        wc = np.asarray(Wm, np.float32)[sl, :].astype(f16)  # [128e, D]
        return wc.reshape(128, ND, 128).transpose(2, 1, 0)   # [p, d, e]

    in_maps = []
    for c in range(N_CORES):
        sl = slice(c * E, (c + 1) * E)
        wq = pack_w(W_Q, sl)
        wk = pack_w(W_K, sl)
        wv = pack_w(W_V, sl)
        in_maps.append({
            **common,
            "wqk": np.ascontiguousarray(
                np.concatenate([wq, wk], axis=1).reshape(128, 2 * ND * 128)),
            "wv": np.ascontiguousarray(wv.reshape(128, ND * 128)),
        })
    return in_maps


def kernel(input, attention_mask, W_Q, W_K, W_V):
    from concourse.bass_utils import run_bass_kernel_spmd

    nc = _get_program()
    in_maps = _prep_inputs(input, attention_mask, W_Q, W_K, W_V)
    res = run_bass_kernel_spmd(nc, in_maps, list(range(N_CORES)))
    return np.concatenate([res.results[c]["out"] for c in range(N_CORES)],
                          axis=2)


# revision 52
# speedup vs baseline: 1.2371x; 1.0406x over previous
"""Trainium2 Bass kernel for sparse (causal, tanh-clamped) attention.

Problem: B=2, L=2048, D=1024, H=16 heads x 64 dim; S = QK^T/8;
S = 30*tanh(S); causal + attention_mask; softmax; out = attn @ V.

Sharding: 2 heads per core across 8 cores (tensor-parallel on heads).

v2 design (ACT is the bottleneck engine: 2 passes tanh+exp over all
causal score columns ~= 116us + per-instruction overheads ~= 150us):
 - bf16 inputs/weights/QKV/probabilities (matmul cost is per moving row
   regardless of dtype>=bf16; halves DMA bytes and removes the fp32r
   <256-row 4x penalty).
 - S^T[k, q] layout throughout; no P transpose (P^T feeds AV directly).
 - attention_mask folded into the score matmul via 65th contraction row.
 - causal diagonal-block mask applied INSIDE the score accumulation as
   one extra matmul (maskT stationary, identity moving): no DVE/ACT cost.
 - bounded logits: P = exp(30*tanh(s)-30), no running max; denominator
   via a ones-column in V (column 0 of the augmented V tile).
 - V is projected token-major directly (stationary=x chunk, moving=W_V
   chunk) -- no V transposes, no separate vaug copies.
 - AV accumulation per 128-column tile with stop at ki==j: epilogues
   drain per-512 bank as soon as its last k-tile lands (spread through
   the run; tiny tail).
 - software-pipelined emission: per k-tile iteration emit score(ki),
   tanh/exp(ki), then AV(ki-1) and budget-limited background quanta
   (projections/V/stores) so the in-order PE queue never starves ACT.
"""

import sys

if "/opt/trn_rl_repo" not in sys.path:
    sys.path.insert(0, "/opt/trn_rl_repo")

import numpy as np

B = 2
L = 2048
D = 1024
H = 16
DH = 64
N_CORES = 8
T = B * L            # 4096 tokens
E = 128              # per-core output features (2 heads)
ND = D // 128        # 8 contraction chunks
NEG_BIG = 6.0e4   # fp16-safe; still saturates tanh
TAU = 30.0

_CACHE = {}


def _build_program():
    import concourse.bacc as bacc
    import concourse.tile as tile
    from concourse import mybir
    from collections import deque

    F32 = mybir.dt.float32
    F16 = mybir.dt.float16
    BF16 = mybir.dt.bfloat16
    AF = mybir.ActivationFunctionType

    nc = bacc.Bacc("TRN2", target_bir_lowering=False, debug=False,
                   num_devices=N_CORES)

    # --- DRAM tensors -----------------------------------------------------
    # x host-packed: group0 of each batch split in two 256-token pieces
    # laid out [p, (d, t)], remaining groups [p, (g, d, t512)].
    xh0a_d = [nc.dram_tensor(f"xh0a{b}", [128, ND * 256], F16,
                             kind="ExternalInput") for b in range(B)]
    xh0b_d = [nc.dram_tensor(f"xh0b{b}", [128, ND * 256], F16,
                             kind="ExternalInput") for b in range(B)]
    xhr_d = nc.dram_tensor("xhr", [128, 6 * ND * 512], F16,
                           kind="ExternalInput")
    wqk_d = nc.dram_tensor("wqk", [128, 2 * ND * 128], F16,
                           kind="ExternalInput")
    wv_d = nc.dram_tensor("wv", [128, ND * 128], F16,
                          kind="ExternalInput")
    kaug_d = nc.dram_tensor("kaug", [1, T], F16, kind="ExternalInput")
    trilb_d = nc.dram_tensor("trilb", [128, 128], BF16,
                             kind="ExternalInput")
    identf_d = nc.dram_tensor("identf", [128, 128], F32,
                              kind="ExternalInput")
    out_d = nc.dram_tensor("out", [B, L, E], F32, kind="ExternalOutput")

    NJ = L // 128     # 16 column tiles per sequence

    with tile.TileContext(nc) as tc:
        with (
            tc.tile_pool(name="const", bufs=1) as constp,
            tc.tile_pool(name="weights", bufs=1) as wp,
            tc.tile_pool(name="qk", bufs=1) as qkp,
            tc.tile_pool(name="va", bufs=1) as vap,
            tc.tile_pool(name="x0", bufs=4) as x0p,
            tc.tile_pool(name="xg", bufs=6) as xgp,
            tc.tile_pool(name="prob", bufs=6) as ppp,
            tc.tile_pool(name="epi", bufs=6) as epip,
            tc.tile_pool(name="ostage", bufs=1) as ostagep,
            tc.tile_pool(name="strip", bufs=3, space="PSUM") as stripp,
            tc.tile_pool(name="psO", bufs=1, space="PSUM") as pop,
        ):
            # --- constants ----------------------------------------------
            trilb_t = constp.tile([128, 128], BF16, tag="trilb")
            identf_t = constp.tile([128, 128], F32, tag="identf")
            n30_t = constp.tile([128, 1], F32, tag="n30")
            wz_t = constp.tile([128, 128], F16, tag="wzero")
            nc.gpsimd.memset(wz_t[:], 0.0)

            wqk_t = wp.tile([128, 2 * ND * 128], F16, tag="wqk",
                            name="wqk")
            wv_t = wp.tile([128, ND * 128], F16, tag="wv", name="wv")

            QT = [[qkp.tile([65, L], F16, tag=f"qt{h}{b}",
                            name=f"qt{h}{b}")
                   for b in range(B)] for h in range(2)]
            KT = [[qkp.tile([65, L], F16, tag=f"kt{h}{b}",
                            name=f"kt{h}{b}")
                   for b in range(B)] for h in range(2)]
            # va[b][ki]: [128 tokens, 131]: col0=ones, 1:65=h0 feats,
            # col65=ones, 66:130=h1 feats
            VA = [[vap.tile([128, 131], BF16, tag=f"va{b}_{k}",
                            name=f"va{b}_{k}") for k in range(NJ)]
                  for b in range(B)]
            OST = [[ostagep.tile([128, 512], F32, tag=f"os{b}_{k}",
                                 name=f"os{b}_{k}") for k in range(4)]
                   for b in range(B)]

            # aug rows for batch 0 first: the first scores read QT
            # row 64, and these [1, 2048] memsets cost ~1.8us each on
            # the serial Pool queue
            for h in range(2):
                nc.gpsimd.memset(QT[h][0][64:65, :], 1.0)
            nc.gpsimd.memset(n30_t[:], -TAU)
            # ones columns of VA (written once; Pool is idle)
            for b in range(B):
                for k in range(NJ):
                    nc.gpsimd.memset(VA[b][k][:, 0:1], 1.0)
                    nc.gpsimd.memset(VA[b][k][:, 65:66], 1.0)
            for h in range(2):
                nc.gpsimd.memset(QT[h][1][64:65, :], 1.0)

            # --- initial DMAs. The startup chain is HWDGE-bound
            # (~625ns per dma_start on a serial device), so the minimum
            # number of transfers gates the first projection.
            x0_tiles = {}   # (b, half) -> tile
            x0_tiles[(0, 0)] = x0p.tile([128, ND * 256], F16, tag="x0",
                                        name="x00")
            nc.sync.dma_start(x0_tiles[(0, 0)][:], xh0a_d[0].ap()[:])
            nc.sync.dma_start(wqk_t[:, 0:1024], wqk_d.ap()[:, 0:1024])
            nc.sync.dma_start(wqk_t[:, 1024:2048],
                              wqk_d.ap()[:, 1024:2048])
            for h in range(2):
                nc.sync.dma_start(KT[h][0][64:65, :], kaug_d.ap()[0:1, 0:L])
            x0_tiles[(0, 1)] = x0p.tile([128, ND * 256], F16, tag="x0",
                                        name="x01")
            nc.sync.dma_start(x0_tiles[(0, 1)][:], xh0b_d[0].ap()[:])
            nc.sync.dma_start(trilb_t[:], trilb_d.ap()[:])
            nc.sync.dma_start(wv_t[:], wv_d.ap()[:])
            nc.sync.dma_start(identf_t[:], identf_d.ap()[:])

            # PE warmup: ramp the p-state and keep PE busy until the
            # first projection inputs land (a gap resets the ramp).
            wm = stripp.tile([128, 1024], F32, tag="strip",
                             name="warm")
            for _ in range(20):
                nc.tensor.matmul(wm[:, 0:128], wz_t[:], wz_t[:],
                                 start=True, stop=True)

            xg_tiles = {}   # group g (1..7) -> tile

            def load_group(g):
                t = xgp.tile([128, ND * 512], F16, tag="xg",
                             name=f"xg{g}")
                idx = g - 1 if g <= 3 else g - 2   # xhr: groups 1,2,3,5,6,7
                nc.sync.dma_start(
                    t[:], xhr_d.ap()[:, idx * 4096:(idx + 1) * 4096])
                xg_tiles[g] = t

            def load_aug_b1():
                for h in range(2):
                    nc.sync.dma_start(KT[h][1][64:65, :],
                                      kaug_d.ap()[0:1, L:2 * L])

            def x_slice(b, t0, width):
                """SBUF source tile for tokens [t0, t0+width) of batch b:
                returns (tile, per-d stride, column base)."""
                tglob = b * L + t0
                g = tglob // 512
                if g in (0, 4):   # each batch's first group: split tiles
                    half = (tglob % 512) // 256
                    return x0_tiles[(b, half)], 256, tglob % 256
                return xg_tiles[g], 512, tglob % 512

            def load_b1_first():
                for half in range(2):
                    t = x0p.tile([128, ND * 256], F16, tag="x0",
                                 name=f"x1{half}")
                    nc.sync.dma_start(t[:], (xh0a_d[1] if half == 0
                                             else xh0b_d[1]).ap()[:])
                    x0_tiles[(1, half)] = t

            # --- projection emitters ------------------------------------
            def qk_chunk_mm(b, c, dlo, dhi, pj):
                """QK projection matmuls for 256-token chunk c of batch
                b, d-chunks [dlo, dhi)."""
                t0 = 256 * c
                xt, tw, base = x_slice(b, t0, 256)
                for d in range(dlo, dhi):
                    xs = xt[:, d * tw + base:d * tw + base + 256]
                    nc.tensor.matmul(
                        pj[:, 0:256], wqk_t[:, d * 128:(d + 1) * 128],
                        xs, start=(d == 0), stop=(d == ND - 1))
                    nc.tensor.matmul(
                        pj[:, 256:512],
                        wqk_t[:, ND * 128 + d * 128:ND * 128 + (d + 1) * 128],
                        xs, start=False, stop=(d == ND - 1))

            def qk_chunk_drain(b, c, part, pj, on_act=False, hs=(0, 1)):
                """Drain chunk c: part 0 = Q, part 1 = K; heads hs."""
                t0 = 256 * c
                dst = QT if part == 0 else KT
                for h in hs:
                    src = pj[h * 64:(h + 1) * 64,
                             part * 256:part * 256 + 256]
                    d = dst[h][b][0:64, t0:t0 + 256]
                    if on_act:
                        nc.scalar.activation(d, src, AF.Identity)
                    else:
                        nc.vector.tensor_copy(d, src)

            def v_tile(b, ki):
                """V projection for token-tile ki of batch b:
                token-major via stationary-x matmul; drains into VA."""
                t0 = 128 * ki
                xt, tw, base = x_slice(b, t0, 128)
                pv = stripp.tile([128, 1024], F32, tag="strip",
                                 name="pv")
                for d in range(ND):
                    nc.tensor.matmul(
                        pv[:, 0:128],
                        xt[:, d * tw + base:d * tw + base + 128],
                        wv_t[:, d * 128:(d + 1) * 128],
                        start=(d == 0), stop=(d == ND - 1))
                nc.vector.tensor_copy(VA[b][ki][:, 1:65], pv[:, 0:64])
                nc.vector.tensor_copy(VA[b][ki][:, 66:130], pv[:, 64:128])

            # --- background queue ---------------------------------------
            bg = deque()
            emitted = set()
            by_key = {}

            def bg_add(key, pe, dve, fn, requires=()):
                it = {"key": key, "pe": pe, "dve": dve, "fn": fn,
                      "req": tuple(requires)}
                bg.append(it)
                by_key[key] = it

            def bg_run(item):
                if item["key"] in emitted:
                    return
                for r in item["req"]:
                    if r not in emitted and r in by_key:
                        bg_run(by_key[r])
                emitted.add(item["key"])
                item["fn"]()

            def pump(pe_budget, dve_budget):
                # scan past blocked items (requirements make out-of-order
                # emission safe); bounded lookahead keeps need-order bias.
                # At most one PSUM-using (PE) quantum per call: a second
                # would cycle the strip ring into a head-of-line stall.
                scanned = 0
                pe_quanta = 0
                i = 0
                while i < len(bg) and scanned < 12:
                    it = bg[i]
                    if it["key"] in emitted:
                        del bg[i]
                        continue
                    scanned += 1
                    fits = (it["pe"] <= pe_budget
                            and it["dve"] <= dve_budget
                            and (it["pe"] == 0 or pe_quanta < 1))
                    if fits:
                        del bg[i]
                        bg_run(it)
                        pe_budget -= it["pe"]
                        dve_budget -= it["dve"]
                        if it["pe"] > 0:
                            pe_quanta += 1
                    else:
                        i += 1

            def force(pred):
                # emit exactly the matching items (plus their declared
                # prerequisites), leaving unrelated queued work in place
                for it in list(bg):
                    if it["key"] not in emitted and pred(it["key"]):
                        bg_run(it)
                while bg and bg[0]["key"] in emitted:
                    bg.popleft()

            def x_req(b, t0):
                g = (b * L + t0) // 512
                if g == 0:
                    return ()
                if g == 4:
                    return (("x1a", 0),)
                return (("xg", g),)

            def queue_qk(b, c, act_drain=False):
                shared = {}
                req = x_req(b, 256 * c)

                def mk_mm(dlo, dhi):
                    def fn():
                        if "pj" not in shared:
                            shared["pj"] = stripp.tile([128, 1024], F32,
                                                       tag="strip",
                                                       name="pj")
                        qk_chunk_mm(b, c, dlo, dhi, shared["pj"])
                    return fn

                def mk_drain(part):
                    def fn():
                        qk_chunk_drain(b, c, part, shared["pj"],
                                       on_act=act_drain)
                    return fn
                for q in range(4):
                    bg_add(("qkm", b, c, q), 440, 0,
                           mk_mm(q * 2, q * 2 + 2),
                           req if q == 0 else (("qkm", b, c, q - 1),))
                bg_add(("qkd", b, c, 0), 0, 790, mk_drain(0),
                       (("qkm", b, c, 3),))
                bg_add(("qkd", b, c, 1), 0, 790, mk_drain(1),
                       (("qkm", b, c, 3),))

            def queue_v(b, ki):
                bg_add(("v", b, ki), 430, 390,
                       lambda b=b, ki=ki: v_tile(b, ki),
                       x_req(b, 128 * ki))

            def queue_dma(key, fn, requires=()):
                bg_add(key, 0, 0, fn, requires)

            # --- attention ----------------------------------------------
            def queue_store(b, blk):
                def fn(b=b, blk=blk):
                    nc.gpsimd.dma_start(
                        out_d.ap()[b, blk * 512:(blk + 1) * 512, :]
                        .rearrange("(j p) e -> p j e", p=128),
                        OST[b][blk][:].rearrange("p (j e) -> p j e", j=4))
                queue_dma(("store", b, blk), fn)

            def normalize_tile(b, h, j, dst):
                """dst: transposed [128, 65] PSUM view (col0 = denom)."""
                rec = epip.tile([128, 1], F32, tag="rec", name="rec")
                nc.vector.reciprocal(rec[:], dst[0:128, 0:1])
                blk, j_in = j // 4, j % 4
                nc.vector.tensor_scalar_mul(
                    OST[b][blk][:, j_in * 128 + h * 64:
                                j_in * 128 + h * 64 + 64],
                    dst[0:128, 1:65], rec[:])
                if h == 1:
                    if b == 1 and blk == 3:
                        # final block: per-tile stores on the sync queue
                        # (HWDGE; the SWDGE gen on Pool would serialize
                        # the last three stores at ~1us each)
                        def fn(b=b, j=j, j_in=j_in, blk=blk):
                            nc.sync.dma_start(
                                out_d.ap()[b, j * 128:(j + 1) * 128, :],
                                OST[b][blk][:, j_in * 128:
                                            (j_in + 1) * 128])
                        queue_dma(("store", b, blk, j), fn)
                    elif j % 4 == 3:
                        queue_store(b, blk)

            def attention_span(b, qlo, qw, last_span=False,
                               carry=None):
                """Merged-head span: one [128, <=1024] strip holds both
                heads' score columns for each k-tile; a single tanh/exp
                pair covers them (halves the ACT instruction count).
                qw <= 512. Head 1's block sits at offset `h1b`:
                packed at w when 2w <= 512, else at 512 (bank B)."""
                jlo, jhi = qlo // 128, (qlo + qw) // 128
                state = {0: None, 1: None}
                prev = None
                bank_ots = {}   # h -> ot tile for the span's bank

                def get_po(h):
                    if state[h] is None:
                        state[h] = pop.tile([65, 512], F32,
                                            tag=f"po{h}", name=f"po{h}")
                    return state[h]

                sc = {}

                def emit_score(ki):
                    q0 = max(qlo, ki * 128)
                    w = qlo + qw - q0
                    h1b = w if 2 * w <= 512 else 512
                    strip = stripp.tile([128, 1024], F32, tag="strip",
                                        name="strip")
                    for h in range(2):
                        nc.tensor.matmul(
                            strip[:, h * h1b:h * h1b + w],
                            KT[h][b][:, ki * 128:ki * 128 + 128],
                            QT[h][b][:, q0:q0 + w],
                            start=True, stop=True)
                    sc[ki] = (strip, q0, w, h1b)

                def emit_av(ki, pp, q0, w, h1b):
                    for h in range(2):
                        po = get_po(h)
                        for j in range(max(ki, jlo), jhi):
                            cl = h * h1b + j * 128 - q0
                            nc.tensor.matmul(
                                po[:, j * 128 - qlo:(j + 1) * 128 - qlo],
                                VA[b][ki][:, h * 65:h * 65 + 65],
                                pp[:, cl:cl + 128],
                                start=(ki == 0 and j == jlo),
                                stop=(ki == j))
                            if ki != j:
                                continue
                            if last_span and j >= jhi - 4:
                                ot = epip.tile([65, 512], F32,
                                               tag="ot", name="ot")
                                nc.vector.tensor_copy(
                                    ot[0:65, 0:128],
                                    po[:, j * 128 - qlo:
                                        (j + 1) * 128 - qlo])
                                pt = stripp.tile([128, 1024], F32,
                                                 tag="strip", name="pt")
                                nc.tensor.transpose(
                                    pt[0:128, 0:65], ot[0:65, 0:128],
                                    identf_t[0:65, 0:65])
                                normalize_tile(b, h, j,
                                               pt[0:128, 0:65])
                            elif j == jhi - 1:
                                nb = jhi - jlo
                                ot = epip.tile([65, 512], F32,
                                               tag="ot", name="ot")
                                nc.vector.tensor_copy(
                                    ot[0:65, 0:nb * 128],
                                    po[:, 0:nb * 128])
                                bank_ots[h] = (ot, jlo, nb)

                emit_score(0)
                for ki in range(jhi):
                    if ki + 1 < jhi:
                        emit_score(ki + 1)
                    strip, q0, w, h1b = sc.pop(ki)
                    diag = (q0 == ki * 128)
                    tot = h1b + w
                    nc.scalar.activation(strip[:, 0:tot],
                                         strip[:, 0:tot],
                                         AF.Tanh, scale=0.125)
                    pp = ppp.tile([128, 1024], BF16, tag="pp",
                                  name="pp")
                    nc.scalar.activation(pp[:, 0:tot], strip[:, 0:tot],
                                         AF.Exp, bias=n30_t[:],
                                         scale=TAU)
                    if diag:
                        # exact-zero the below-diagonal ghosts
                        nc.vector.tensor_mul(pp[:, 0:128],
                                             pp[:, 0:128], trilb_t[:])
                        nc.vector.tensor_mul(
                            pp[:, h1b:h1b + 128],
                            pp[:, h1b:h1b + 128], trilb_t[:])
                    if carry is not None:
                        carry()
                        carry = None
                    if prev is not None:
                        if not ("v", b, prev[0]) in emitted:
                            force(lambda k, kk=prev[0]:
                                  k == ("v", b, kk))
                        emit_av(*prev)
                    act_ns = 1.67 * tot + 330
                    pe_ns = 0.43 * tot + (120 if diag else 0) + \
                        0.86 * 128 * max(0, jhi - max(ki - 1, jlo)) + 100
                    pump(max(0.0, act_ns - pe_ns - 100),
                         max(0.0, act_ns - 550))
                    prev = (ki, pp, q0, w, h1b)
                if prev is not None:
                    if not ("v", b, prev[0]) in emitted:
                        force(lambda k, kk=prev[0]: k == ("v", b, kk))
                    emit_av(*prev)

                def finish():
                    for h in sorted(bank_ots):
                        ot, jb, nb = bank_ots[h]
                        for jj in range(nb):
                            pt = pop.tile([128, 65], F32, tag="po0",
                                          name="pt")
                            nc.tensor.transpose(
                                pt[:], ot[0:65, jj * 128:(jj + 1) * 128],
                                identf_t[0:65, 0:65])
                            normalize_tile(b, h, jb + jj, pt[:])
                return finish

            # --- orchestration ------------------------------------------
            # Spans alternate heads: h1 re-uses h0's projections, so the
            # ACT work per projection deadline doubles and background
            # projection quanta fit inside the span slack.
            # prologue: QK chunk 0 + V tiles 0-1 of b0 (drains split
            # ACT/DVE to shorten the startup chain)
            pj0 = stripp.tile([128, 1024], F32, tag="strip",
                              name="pj0")
            qk_chunk_mm(0, 0, 0, ND, pj0)
            # h0's Q/K drains on ACT, h1's on DVE: they run in parallel
            # and the merged first span needs all four
            qk_chunk_drain(0, 0, 0, pj0, on_act=True, hs=(0,))
            qk_chunk_drain(0, 0, 1, pj0, on_act=True, hs=(0,))
            qk_chunk_drain(0, 0, 0, pj0, on_act=False, hs=(1,))
            qk_chunk_drain(0, 0, 1, pj0, on_act=False, hs=(1,))
            for ki in (0, 1):
                queue_v(0, ki)


            for g in (1, 2, 3):
                queue_dma(("xg", g), lambda g=g: load_group(g))
            for c in (2, 3):
                queue_qk(0, c)
            for ki in (2, 3, 4, 5):
                queue_v(0, ki)
            for c in (4, 5):
                queue_qk(0, c)
            for ki in (6, 7, 8, 9):
                queue_v(0, ki)
            for c in (6, 7):
                queue_qk(0, c)
            for ki in range(10, NJ):
                queue_v(0, ki)

            def need_qk(b, cs):
                force(lambda k: k[0] in ("qkm", "qkd")
                      and k[1] == b and k[2] in cs)

            def tk(*keys):
                return deque(keys)

            def chunk_keys(b, *cs):
                out = []
                for c in cs:
                    out += [("qkm", b, c, q) for q in range(4)]
                    out += [("qkd", b, c, p, hh)
                            for p in range(2) for hh in range(2)]
                return deque(out)

            # batch 0 (merged-head spans, qw <= 512)
            cr = attention_span(0, 0, 256)
            # chunk 1 matmuls inline right after span 1's emission (they
            # run during its ACT work); drains via the background queue
            pj1 = stripp.tile([128, 1024], F32, tag="strip",
                              name="pj1")
            qk_chunk_mm(0, 1, 0, ND, pj1)
            for part in range(2):
                bg_add(("qkd", 0, 1, part), 0, 790,
                       lambda p=part: qk_chunk_drain(0, 1, p, pj1))
            for q in range(4):
                emitted.add(("qkm", 0, 1, q))
            force(lambda k: k[0] == "qkd" and k[1] == 0 and k[2] == 1)
            cr = attention_span(0, 256, 256, carry=cr)
            need_qk(0, (2, 3))
            cr = attention_span(0, 512, 512, carry=cr)
            # queue b1 inputs + b1 projection work mid-flight
            queue_dma(("x1a", 0), load_b1_first)
            queue_dma(("aug1", 0), load_aug_b1)
            for g in (5, 6, 7):
                queue_dma(("xg", g), lambda g=g: load_group(g))
            for c in (0, 1, 2, 3):
                queue_qk(1, c)
            for ki in (0, 1, 2, 3):
                queue_v(1, ki)
            for c in (4, 5, 6, 7):
                queue_qk(1, c)
            for ki in range(4, NJ):
                queue_v(1, ki)
            need_qk(0, (4, 5))
            cr = attention_span(0, 1024, 512, carry=cr)
            need_qk(0, (6, 7))
            cr = attention_span(0, 1536, 512, carry=cr)

            # batch 1
            force(lambda k: k[0] in ("x1a", "aug1"))
            need_qk(1, (0, 1))
            cr = attention_span(1, 0, 512, carry=cr)
            need_qk(1, (2, 3))
            cr = attention_span(1, 512, 512, carry=cr)
            need_qk(1, (4, 5))
            cr = attention_span(1, 1024, 512, carry=cr)
            need_qk(1, (6, 7))
            cr = attention_span(1, 1536, 512, last_span=True, carry=cr)
            cr()
            force(lambda k: True)

    nc.compile()
    return nc


def _get_program():
    if "nc" not in _CACHE:
        _CACHE["nc"] = _build_program()
    return _CACHE["nc"]


def _prep_inputs(input, attention_mask, W_Q, W_K, W_V):
    f16 = np.float16

    x = np.asarray(input, dtype=np.float32).reshape(T, D)
    xT = np.ascontiguousarray(x.T).astype(f16)             # [D, T]
    # [d, p, g, tloc]
    xr = xT.reshape(ND, 128, B * 4, 512).transpose(1, 2, 0, 3)
    # xr: [p, g, d, tloc]
    xh = {}
    for b in range(B):
        g0 = xr[:, b * 4]                                    # [p, d, 512]
        xh[f"xh0a{b}"] = np.ascontiguousarray(
            g0[:, :, 0:256].reshape(128, ND * 256))
        xh[f"xh0b{b}"] = np.ascontiguousarray(
            g0[:, :, 256:512].reshape(128, ND * 256))
    rest = np.concatenate([xr[:, g] for g in (1, 2, 3, 5, 6, 7)],
                          axis=1)                            # [p, 6*d, 512]
    xhr = np.ascontiguousarray(rest.reshape(128, 6 * ND * 512))

    import ml_dtypes
    mask = np.asarray(attention_mask).astype(np.float32).reshape(1, T)
    kaug = ((mask - 1.0) * NEG_BIG).astype(f16)
    qi = np.arange(128)
    trilb = np.where(qi[None, :] >= qi[:, None], 1.0,
                     0.0).astype(ml_dtypes.bfloat16)   # keep[k,q]: q >= k
    identf = np.eye(128, dtype=np.float32)

    common = {
        **xh, "xhr": xhr, "kaug": kaug, "trilb": trilb,
        "identf": identf,
    }

    def pack_w(Wm, sl):
        wc = np.asarray(Wm, np.float32)[sl, :].astype(f16)  # [128e, D]
        return wc.reshape(128, ND, 128).transpose(2, 1, 0)   # [p, d, e]

    in_maps = []
    for c in range(N_CORES):
        sl = slice(c * E, (c + 1) * E)
        wq = pack_w(W_Q, sl)
        wk = pack_w(W_K, sl)
        wv = pack_w(W_V, sl)
        in_maps.append({
            **common,
            "wqk": np.ascontiguousarray(
                np.concatenate([wq, wk], axis=1).reshape(128, 2 * ND * 128)),
            "wv": np.ascontiguousarray(wv.reshape(128, ND * 128)),
        })
    return in_maps


def kernel(input, attention_mask, W_Q, W_K, W_V):
    from concourse.bass_utils import run_bass_kernel_spmd

    nc = _get_program()
    in_maps = _prep_inputs(input, attention_mask, W_Q, W_K, W_V)
    res = run_bass_kernel_spmd(nc, in_maps, list(range(N_CORES)))
    return np.concatenate([res.results[c]["out"] for c in range(N_CORES)],
                          axis=2)


# revision 56
# speedup vs baseline: 1.2566x; 1.0158x over previous
"""Trainium2 Bass kernel for sparse (causal, tanh-clamped) attention.

Problem: B=2, L=2048, D=1024, H=16 heads x 64 dim; S = QK^T/8;
S = 30*tanh(S); causal + attention_mask; softmax; out = attn @ V.

Sharding: 2 heads per core across 8 cores (tensor-parallel on heads).

The ACT (scalar) engine is the bottleneck: tanh+exp over every causal
score column is ~116us of irreducible element time; everything else is
arranged to keep ACT saturated and to minimize its per-instruction
overhead (~330ns per tanh/exp pair).

Design:
 - fp16 x/W/Q/K (precision for the score path: the exp amplifies score
   errors by d(30*tanh)/ds), bf16 probabilities/V (P can be ~e^-60;
   fp16 would flush it to 0 and NaN the softmax denominator). All
   matmuls cost 1 cycle/moving-row at 16-bit; DMA bytes are halved.
 - S^T[k, q] layout throughout; no P transpose (P^T feeds AV directly).
 - attention_mask folded into the score matmul via a 65th contraction
   row; causal diagonal masked by a tril MULTIPLY on the probabilities
   (an additive -big mask would leave e^-60 ghosts that dominate rows
   whose live probabilities are comparably small).
 - bounded logits: P = exp(30*tanh(s)-30), no running max; denominator
   comes free as a ones-column in the augmented V tile (po row 0).
 - V projected token-major directly (stationary=x chunk, moving=W_V
   chunk): no V transposes, no separate vaug copies.
 - merged-head spans (qw<=512): one [128, <=1024] PSUM strip holds both
   heads' score columns per k-tile, so a single tanh/exp pair covers
   both heads -- 82 ACT pairs total instead of 102.
 - AV accumulation per 128-column tile with stop at ki==j; bank drains
   to SBUF when retired, transpose/normalize chain deferred into the
   next span (psO-ring aliasing requires pts after po's last access).
   The last span drains per-128 with pt from the strip ring and
   per-tile stores on the sync queue for a ~4us tail.
 - software-pipelined emission: score(ki+1) prefetched ahead of AV/pump
   on the in-order PE queue; background work (projection chunks, V
   tiles, x loads, stores) flows through a budget-limited pump with
   explicit prerequisites, forced just-in-time at span boundaries with
   drains split across ACT/DVE when ACT would be idle anyway.
"""

import sys

if "/opt/trn_rl_repo" not in sys.path:
    sys.path.insert(0, "/opt/trn_rl_repo")

import numpy as np

B = 2
L = 2048
D = 1024
H = 16
DH = 64
N_CORES = 8
T = B * L            # 4096 tokens
E = 128              # per-core output features (2 heads)
ND = D // 128        # 8 contraction chunks
NEG_BIG = 6.0e4   # fp16-safe; still saturates tanh
TAU = 30.0

_CACHE = {}


def _build_program():
    import concourse.bacc as bacc
    import concourse.tile as tile
    from concourse import mybir
    from collections import deque

    F32 = mybir.dt.float32
    F16 = mybir.dt.float16
    BF16 = mybir.dt.bfloat16
    AF = mybir.ActivationFunctionType

    nc = bacc.Bacc("TRN2", target_bir_lowering=False, debug=False,
                   num_devices=N_CORES)

    # --- DRAM tensors -----------------------------------------------------
    # x host-packed: group0 of each batch split in two 256-token pieces
    # laid out [p, (d, t)], remaining groups [p, (g, d, t512)].
    xh0a_d = [nc.dram_tensor(f"xh0a{b}", [128, ND * 256], F16,
                             kind="ExternalInput") for b in range(B)]
    xh0b_d = [nc.dram_tensor(f"xh0b{b}", [128, ND * 256], F16,
                             kind="ExternalInput") for b in range(B)]
    xhr_d = nc.dram_tensor("xhr", [128, 6 * ND * 512], F16,
                           kind="ExternalInput")
    wqk_d = nc.dram_tensor("wqk", [128, 2 * ND * 128], F16,
                           kind="ExternalInput")
    wv_d = nc.dram_tensor("wv", [128, ND * 128], F16,
                          kind="ExternalInput")
    kaug_d = nc.dram_tensor("kaug", [1, T], F16, kind="ExternalInput")
    trilb_d = nc.dram_tensor("trilb", [128, 128], BF16,
                             kind="ExternalInput")
    identf_d = nc.dram_tensor("identf", [128, 128], F32,
                              kind="ExternalInput")
    out_d = nc.dram_tensor("out", [B, L, E], F32, kind="ExternalOutput")

    NJ = L // 128     # 16 column tiles per sequence

    with tile.TileContext(nc) as tc:
        with (
            tc.tile_pool(name="const", bufs=1) as constp,
            tc.tile_pool(name="weights", bufs=1) as wp,
            tc.tile_pool(name="qk", bufs=1) as qkp,
            tc.tile_pool(name="va", bufs=1) as vap,
            tc.tile_pool(name="x0", bufs=4) as x0p,
            tc.tile_pool(name="xg", bufs=6) as xgp,
            tc.tile_pool(name="prob", bufs=6) as ppp,
            tc.tile_pool(name="epi", bufs=6) as epip,
            tc.tile_pool(name="ostage", bufs=1) as ostagep,
            tc.tile_pool(name="strip", bufs=3, space="PSUM") as stripp,
            tc.tile_pool(name="psO", bufs=1, space="PSUM") as pop,
        ):
            # --- constants ----------------------------------------------
            trilb_t = constp.tile([128, 128], BF16, tag="trilb")
            identf_t = constp.tile([128, 128], F32, tag="identf")
            n30_t = constp.tile([128, 1], F32, tag="n30")
            wz_t = constp.tile([128, 128], F16, tag="wzero")
            nc.gpsimd.memset(wz_t[:], 0.0)

            wqk_t = wp.tile([128, 2 * ND * 128], F16, tag="wqk",
                            name="wqk")
            wv_t = wp.tile([128, ND * 128], F16, tag="wv", name="wv")

            QT = [[qkp.tile([65, L], F16, tag=f"qt{h}{b}",
                            name=f"qt{h}{b}")
                   for b in range(B)] for h in range(2)]
            KT = [[qkp.tile([65, L], F16, tag=f"kt{h}{b}",
                            name=f"kt{h}{b}")
                   for b in range(B)] for h in range(2)]
            # va[b][ki]: [128 tokens, 131]: col0=ones, 1:65=h0 feats,
            # col65=ones, 66:130=h1 feats
            VA = [[vap.tile([128, 131], BF16, tag=f"va{b}_{k}",
                            name=f"va{b}_{k}") for k in range(NJ)]
                  for b in range(B)]
            OST = [[ostagep.tile([128, 512], F32, tag=f"os{b}_{k}",
                                 name=f"os{b}_{k}") for k in range(4)]
                   for b in range(B)]

            # aug rows for batch 0 first: the first scores read QT
            # row 64, and these [1, 2048] memsets cost ~1.8us each on
            # the serial Pool queue
            for h in range(2):
                nc.gpsimd.memset(QT[h][0][64:65, :], 1.0)
            nc.gpsimd.memset(n30_t[:], -TAU)
            # ones columns of VA (written once; Pool is idle)
            for b in range(B):
                for k in range(NJ):
                    nc.gpsimd.memset(VA[b][k][:, 0:1], 1.0)
                    nc.gpsimd.memset(VA[b][k][:, 65:66], 1.0)
            for h in range(2):
                nc.gpsimd.memset(QT[h][1][64:65, :], 1.0)

            # --- initial DMAs. The startup chain is HWDGE-bound
            # (~625ns per dma_start on a serial device), so the minimum
            # number of transfers gates the first projection.
            x0_tiles = {}   # (b, half) -> tile
            x0_tiles[(0, 0)] = x0p.tile([128, ND * 256], F16, tag="x0",
                                        name="x00")
            nc.sync.dma_start(x0_tiles[(0, 0)][:], xh0a_d[0].ap()[:])
            nc.sync.dma_start(wqk_t[:, 0:1024], wqk_d.ap()[:, 0:1024])
            nc.sync.dma_start(wqk_t[:, 1024:2048],
                              wqk_d.ap()[:, 1024:2048])
            for h in range(2):
                nc.sync.dma_start(KT[h][0][64:65, :], kaug_d.ap()[0:1, 0:L])
            x0_tiles[(0, 1)] = x0p.tile([128, ND * 256], F16, tag="x0",
                                        name="x01")
            nc.sync.dma_start(x0_tiles[(0, 1)][:], xh0b_d[0].ap()[:])
            nc.sync.dma_start(trilb_t[:], trilb_d.ap()[:])
            nc.sync.dma_start(wv_t[:], wv_d.ap()[:])
            nc.sync.dma_start(identf_t[:], identf_d.ap()[:])

            # PE warmup: ramp the p-state and keep PE busy until the
            # first projection inputs land (a gap resets the ramp).
            wm = stripp.tile([128, 1024], F32, tag="strip",
                             name="warm")
            for _ in range(20):
                nc.tensor.matmul(wm[:, 0:128], wz_t[:], wz_t[:],
                                 start=True, stop=True)

            xg_tiles = {}   # group g (1..7) -> tile

            def load_group(g):
                t = xgp.tile([128, ND * 512], F16, tag="xg",
                             name=f"xg{g}")
                idx = g - 1 if g <= 3 else g - 2   # xhr: groups 1,2,3,5,6,7
                nc.sync.dma_start(
                    t[:], xhr_d.ap()[:, idx * 4096:(idx + 1) * 4096])
                xg_tiles[g] = t

            def load_aug_b1():
                for h in range(2):
                    nc.sync.dma_start(KT[h][1][64:65, :],
                                      kaug_d.ap()[0:1, L:2 * L])

            def x_slice(b, t0, width):
                """SBUF source tile for tokens [t0, t0+width) of batch b:
                returns (tile, per-d stride, column base)."""
                tglob = b * L + t0
                g = tglob // 512
                if g in (0, 4):   # each batch's first group: split tiles
                    half = (tglob % 512) // 256
                    return x0_tiles[(b, half)], 256, tglob % 256
                return xg_tiles[g], 512, tglob % 512

            def load_b1_first():
                for half in range(2):
                    t = x0p.tile([128, ND * 256], F16, tag="x0",
                                 name=f"x1{half}")
                    nc.sync.dma_start(t[:], (xh0a_d[1] if half == 0
                                             else xh0b_d[1]).ap()[:])
                    x0_tiles[(1, half)] = t

            # --- projection emitters ------------------------------------
            def qk_chunk_mm(b, c, dlo, dhi, pj):
                """QK projection matmuls for 256-token chunk c of batch
                b, d-chunks [dlo, dhi)."""
                t0 = 256 * c
                xt, tw, base = x_slice(b, t0, 256)
                for d in range(dlo, dhi):
                    xs = xt[:, d * tw + base:d * tw + base + 256]
                    nc.tensor.matmul(
                        pj[:, 0:256], wqk_t[:, d * 128:(d + 1) * 128],
                        xs, start=(d == 0), stop=(d == ND - 1))
                    nc.tensor.matmul(
                        pj[:, 256:512],
                        wqk_t[:, ND * 128 + d * 128:ND * 128 + (d + 1) * 128],
                        xs, start=False, stop=(d == ND - 1))

            def qk_chunk_drain(b, c, part, pj, on_act=False, hs=(0, 1)):
                """Drain chunk c: part 0 = Q, part 1 = K; heads hs."""
                t0 = 256 * c
                dst = QT if part == 0 else KT
                for h in hs:
                    src = pj[h * 64:(h + 1) * 64,
                             part * 256:part * 256 + 256]
                    d = dst[h][b][0:64, t0:t0 + 256]
                    if on_act:
                        nc.scalar.activation(d, src, AF.Identity)
                    else:
                        nc.vector.tensor_copy(d, src)

            def v_tile(b, ki):
                """V projection for token-tile ki of batch b:
                token-major via stationary-x matmul; drains into VA."""
                t0 = 128 * ki
                xt, tw, base = x_slice(b, t0, 128)
                pv = stripp.tile([128, 1024], F32, tag="strip",
                                 name="pv")
                for d in range(ND):
                    nc.tensor.matmul(
                        pv[:, 0:128],
                        xt[:, d * tw + base:d * tw + base + 128],
                        wv_t[:, d * 128:(d + 1) * 128],
                        start=(d == 0), stop=(d == ND - 1))
                nc.vector.tensor_copy(VA[b][ki][:, 1:65], pv[:, 0:64])
                nc.vector.tensor_copy(VA[b][ki][:, 66:130], pv[:, 64:128])

            # --- background queue ---------------------------------------
            bg = deque()
            emitted = set()
            by_key = {}

            def bg_add(key, pe, dve, fn, requires=()):
                it = {"key": key, "pe": pe, "dve": dve, "fn": fn,
                      "req": tuple(requires)}
                bg.append(it)
                by_key[key] = it

            def bg_run(item):
                if item["key"] in emitted:
                    return
                for r in item["req"]:
                    if r not in emitted and r in by_key:
                        bg_run(by_key[r])
                emitted.add(item["key"])
                item["fn"]()

            def pump(pe_budget, dve_budget):
                # scan past blocked items (requirements make out-of-order
                # emission safe); bounded lookahead keeps need-order bias.
                # At most one PSUM-using (PE) quantum per call: a second
                # would cycle the strip ring into a head-of-line stall.
                scanned = 0
                pe_quanta = 0
                i = 0
                while i < len(bg) and scanned < 12:
                    it = bg[i]
                    if it["key"] in emitted:
                        del bg[i]
                        continue
                    scanned += 1
                    fits = (it["pe"] <= pe_budget
                            and it["dve"] <= dve_budget
                            and (it["pe"] == 0 or pe_quanta < 1))
                    if fits:
                        del bg[i]
                        bg_run(it)
                        pe_budget -= it["pe"]
                        dve_budget -= it["dve"]
                        if it["pe"] > 0:
                            pe_quanta += 1
                    else:
                        i += 1

            def force(pred):
                # emit exactly the matching items (plus their declared
                # prerequisites), leaving unrelated queued work in place
                for it in list(bg):
                    if it["key"] not in emitted and pred(it["key"]):
                        bg_run(it)
                while bg and bg[0]["key"] in emitted:
                    bg.popleft()

            def x_req(b, t0):
                g = (b * L + t0) // 512
                if g == 0:
                    return ()
                if g == 4:
                    return (("x1a", 0),)
                return (("xg", g),)

            drain_fns = {}

            def queue_qk(b, c, act_drain=False):
                shared = {}
                req = x_req(b, 256 * c)

                def mk_mm(dlo, dhi):
                    def fn():
                        if "pj" not in shared:
                            shared["pj"] = stripp.tile([128, 1024], F32,
                                                       tag="strip",
                                                       name="pj")
                        qk_chunk_mm(b, c, dlo, dhi, shared["pj"])
                    return fn

                def mk_drain(part, h):
                    def fn(on_act=False):
                        qk_chunk_drain(b, c, part, shared["pj"],
                                       on_act=on_act, hs=(h,))
                    return fn
                for q in range(4):
                    bg_add(("qkm", b, c, q), 440, 0,
                           mk_mm(q * 2, q * 2 + 2),
                           req if q == 0 else (("qkm", b, c, q - 1),))
                for part in range(2):
                    for h in range(2):
                        fn = mk_drain(part, h)
                        drain_fns[(b, c, part, h)] = fn
                        bg_add(("qkd", b, c, part, h), 0, 400, fn,
                               (("qkm", b, c, 3),))

            def queue_v(b, ki):
                bg_add(("v", b, ki), 430, 390,
                       lambda b=b, ki=ki: v_tile(b, ki),
                       x_req(b, 128 * ki))

            def queue_dma(key, fn, requires=()):
                bg_add(key, 0, 0, fn, requires)

            # --- attention ----------------------------------------------
            def queue_store(b, blk):
                def fn(b=b, blk=blk):
                    nc.gpsimd.dma_start(
                        out_d.ap()[b, blk * 512:(blk + 1) * 512, :]
                        .rearrange("(j p) e -> p j e", p=128),
                        OST[b][blk][:].rearrange("p (j e) -> p j e", j=4))
                queue_dma(("store", b, blk), fn)

            def normalize_tile(b, h, j, dst):
                """dst: transposed [128, 65] PSUM view (col0 = denom)."""
                rec = epip.tile([128, 1], F32, tag="rec", name="rec")
                nc.vector.reciprocal(rec[:], dst[0:128, 0:1])
                blk, j_in = j // 4, j % 4
                nc.vector.tensor_scalar_mul(
                    OST[b][blk][:, j_in * 128 + h * 64:
                                j_in * 128 + h * 64 + 64],
                    dst[0:128, 1:65], rec[:])
                if h == 1:
                    if b == 1 and blk == 3:
                        # final block: per-tile stores on the sync queue
                        # (HWDGE; the SWDGE gen on Pool would serialize
                        # the last three stores at ~1us each)
                        def fn(b=b, j=j, j_in=j_in, blk=blk):
                            nc.sync.dma_start(
                                out_d.ap()[b, j * 128:(j + 1) * 128, :],
                                OST[b][blk][:, j_in * 128:
                                            (j_in + 1) * 128])
                        queue_dma(("store", b, blk, j), fn)
                    elif j % 4 == 3:
                        queue_store(b, blk)

            def attention_span(b, qlo, qw, last_span=False,
                               carry=None):
                """Merged-head span: one [128, <=1024] strip holds both
                heads' score columns for each k-tile; a single tanh/exp
                pair covers them (halves the ACT instruction count).
                qw <= 512. Head 1's block sits at offset `h1b`:
                packed at w when 2w <= 512, else at 512 (bank B)."""
                jlo, jhi = qlo // 128, (qlo + qw) // 128
                state = {0: None, 1: None}
                prev = None
                bank_ots = {}   # h -> ot tile for the span's bank

                def get_po(h):
                    if state[h] is None:
                        state[h] = pop.tile([65, 512], F32,
                                            tag=f"po{h}", name=f"po{h}")
                    return state[h]

                sc = {}

                def emit_score(ki):
                    q0 = max(qlo, ki * 128)
                    w = qlo + qw - q0
                    h1b = w if 2 * w <= 512 else 512
                    strip = stripp.tile([128, 1024], F32, tag="strip",
                                        name="strip")
                    for h in range(2):
                        nc.tensor.matmul(
                            strip[:, h * h1b:h * h1b + w],
                            KT[h][b][:, ki * 128:ki * 128 + 128],
                            QT[h][b][:, q0:q0 + w],
                            start=True, stop=True)
                    sc[ki] = (strip, q0, w, h1b)

                def emit_av(ki, pp, q0, w, h1b):
                    for h in range(2):
                        po = get_po(h)
                        for j in range(max(ki, jlo), jhi):
                            cl = h * h1b + j * 128 - q0
                            nc.tensor.matmul(
                                po[:, j * 128 - qlo:(j + 1) * 128 - qlo],
                                VA[b][ki][:, h * 65:h * 65 + 65],
                                pp[:, cl:cl + 128],
                                start=(ki == 0 and j == jlo),
                                stop=(ki == j))
                            if ki != j:
                                continue
                            if last_span and j >= jhi - 4:
                                ot = epip.tile([65, 512], F32,
                                               tag="ot", name="ot")
                                nc.vector.tensor_copy(
                                    ot[0:65, 0:128],
                                    po[:, j * 128 - qlo:
                                        (j + 1) * 128 - qlo])
                                pt = stripp.tile([128, 1024], F32,
                                                 tag="strip", name="pt")
                                nc.tensor.transpose(
                                    pt[0:128, 0:65], ot[0:65, 0:128],
                                    identf_t[0:65, 0:65])
                                normalize_tile(b, h, j,
                                               pt[0:128, 0:65])
                            elif j == jhi - 1:
                                nb = jhi - jlo
                                ot = epip.tile([65, 512], F32,
                                               tag="ot", name="ot")
                                nc.vector.tensor_copy(
                                    ot[0:65, 0:nb * 128],
                                    po[:, 0:nb * 128])
                                bank_ots[h] = (ot, jlo, nb)

                emit_score(0)
                for ki in range(jhi):
                    if ki + 1 < jhi:
                        emit_score(ki + 1)
                    strip, q0, w, h1b = sc.pop(ki)
                    diag = (q0 == ki * 128)
                    tot = h1b + w
                    nc.scalar.activation(strip[:, 0:tot],
                                         strip[:, 0:tot],
                                         AF.Tanh, scale=0.125)
                    pp = ppp.tile([128, 1024], BF16, tag="pp",
                                  name="pp")
                    nc.scalar.activation(pp[:, 0:tot], strip[:, 0:tot],
                                         AF.Exp, bias=n30_t[:],
                                         scale=TAU)
                    if diag:
                        # exact-zero the below-diagonal ghosts
                        nc.vector.tensor_mul(pp[:, 0:128],
                                             pp[:, 0:128], trilb_t[:])
                        nc.vector.tensor_mul(
                            pp[:, h1b:h1b + 128],
                            pp[:, h1b:h1b + 128], trilb_t[:])
                    if carry is not None:
                        carry()
                        carry = None
                    if prev is not None:
                        if not ("v", b, prev[0]) in emitted:
                            force(lambda k, kk=prev[0]:
                                  k == ("v", b, kk))
                        emit_av(*prev)
                    act_ns = 1.67 * tot + 330
                    pe_ns = 0.43 * tot + (120 if diag else 0) + \
                        0.86 * 128 * max(0, jhi - max(ki - 1, jlo)) + 100
                    pump(max(0.0, act_ns - pe_ns - 100),
                         max(0.0, act_ns - 550))
                    prev = (ki, pp, q0, w, h1b)
                if prev is not None:
                    if not ("v", b, prev[0]) in emitted:
                        force(lambda k, kk=prev[0]: k == ("v", b, kk))
                    emit_av(*prev)

                def finish():
                    for h in sorted(bank_ots):
                        ot, jb, nb = bank_ots[h]
                        for jj in range(nb):
                            pt = pop.tile([128, 65], F32, tag="po0",
                                          name="pt")
                            nc.tensor.transpose(
                                pt[:], ot[0:65, jj * 128:(jj + 1) * 128],
                                identf_t[0:65, 0:65])
                            normalize_tile(b, h, jb + jj, pt[:])
                return finish

            # --- orchestration ------------------------------------------
            # Spans alternate heads: h1 re-uses h0's projections, so the
            # ACT work per projection deadline doubles and background
            # projection quanta fit inside the span slack.
            # prologue: QK chunk 0 + V tiles 0-1 of b0 (drains split
            # ACT/DVE to shorten the startup chain)
            pj0 = stripp.tile([128, 1024], F32, tag="strip",
                              name="pj0")
            qk_chunk_mm(0, 0, 0, ND, pj0)
            qk_chunk_drain(0, 0, 0, pj0, on_act=True, hs=(0,))
            qk_chunk_drain(0, 0, 1, pj0, on_act=True, hs=(0,))
            qk_chunk_drain(0, 0, 0, pj0, on_act=False, hs=(1,))
            qk_chunk_drain(0, 0, 1, pj0, on_act=False, hs=(1,))
            for ki in (0, 1):
                queue_v(0, ki)

            # chunk 1 matmuls inline during the ACT-idle startup;
            # its drains go to the background queue
            pj1 = stripp.tile([128, 1024], F32, tag="strip",
                              name="pj1")
            qk_chunk_mm(0, 1, 0, ND, pj1)
            for part in range(2):
                for h in range(2):
                    def c1drain(on_act=False, p=part, hh=h):
                        qk_chunk_drain(0, 1, p, pj1, on_act=on_act,
                                       hs=(hh,))
                    drain_fns[(0, 1, part, h)] = c1drain
                    bg_add(("qkd", 0, 1, part, h), 0, 400, c1drain)
            for q in range(4):
                emitted.add(("qkm", 0, 1, q))


            for g in (1, 2, 3):
                queue_dma(("xg", g), lambda g=g: load_group(g))
            for c in (2, 3):
                queue_qk(0, c)
            for ki in (2, 3, 4, 5):
                queue_v(0, ki)
            for c in (4, 5):
                queue_qk(0, c)
            for ki in (6, 7, 8, 9):
                queue_v(0, ki)
            for c in (6, 7):
                queue_qk(0, c)
            for ki in range(10, NJ):
                queue_v(0, ki)

            def need_qk(b, cs):
                force(lambda k: k[0] == "qkm" and k[1] == b
                      and k[2] in cs)
                # bulk-forced drains: ACT is idle while it waits for
                # these, so alternate them between ACT and DVE
                n = 0
                for c in cs:
                    for part in range(2):
                        for h in range(2):
                            key = ("qkd", b, c, part, h)
                            if key in emitted or key not in by_key:
                                continue
                            emitted.add(key)
                            drain_fns[(b, c, part, h)](on_act=n % 2 == 0)
                            n += 1

            def tk(*keys):
                return deque(keys)

            def chunk_keys(b, *cs):
                out = []
                for c in cs:
                    out += [("qkm", b, c, q) for q in range(4)]
                    out += [("qkd", b, c, p, hh)
                            for p in range(2) for hh in range(2)]
                return deque(out)

            # batch 0 (merged-head spans, qw <= 512)
            cr = attention_span(0, 0, 256)
            need_qk(0, (1,))
            cr = attention_span(0, 256, 256, carry=cr)
            need_qk(0, (2, 3))
            cr = attention_span(0, 512, 512, carry=cr)
            # queue b1 inputs + b1 projection work mid-flight
            queue_dma(("x1a", 0), load_b1_first)
            queue_dma(("aug1", 0), load_aug_b1)
            for g in (5, 6, 7):
                queue_dma(("xg", g), lambda g=g: load_group(g))
            for c in (0, 1, 2, 3):
                queue_qk(1, c)
            for ki in (0, 1, 2, 3):
                queue_v(1, ki)
            for c in (4, 5, 6, 7):
                queue_qk(1, c)
            for ki in range(4, NJ):
                queue_v(1, ki)
            need_qk(0, (4, 5))
            cr = attention_span(0, 1024, 512, carry=cr)
            need_qk(0, (6, 7))
            cr = attention_span(0, 1536, 512, carry=cr)

            # batch 1
            force(lambda k: k[0] in ("x1a", "aug1"))
            need_qk(1, (0, 1))
            cr = attention_span(1, 0, 512, carry=cr)
            need_qk(1, (2, 3))
            cr = attention_span(1, 512, 512, carry=cr)
            need_qk(1, (4, 5))
            cr = attention_span(1, 1024, 512, carry=cr)
            need_qk(1, (6, 7))
            cr = attention_span(1, 1536, 512, last_span=True, carry=cr)
            cr()
            force(lambda k: True)

    nc.compile()
    return nc


def _get_program():
    if "nc" not in _CACHE:
        _CACHE["nc"] = _build_program()
    return _CACHE["nc"]


def _prep_inputs(input, attention_mask, W_Q, W_K, W_V):
    f16 = np.float16

    x = np.asarray(input, dtype=np.float32).reshape(T, D)
    xT = np.ascontiguousarray(x.T).astype(f16)             # [D, T]
    # [d, p, g, tloc]
    xr = xT.reshape(ND, 128, B * 4, 512).transpose(1, 2, 0, 3)
    # xr: [p, g, d, tloc]
    xh = {}
    for b in range(B):
        g0 = xr[:, b * 4]                                    # [p, d, 512]
        xh[f"xh0a{b}"] = np.ascontiguousarray(
            g0[:, :, 0:256].reshape(128, ND * 256))
        xh[f"xh0b{b}"] = np.ascontiguousarray(
            g0[:, :, 256:512].reshape(128, ND * 256))
    rest = np.concatenate([xr[:, g] for g in (1, 2, 3, 5, 6, 7)],
                          axis=1)                            # [p, 6*d, 512]
    xhr = np.ascontiguousarray(rest.reshape(128, 6 * ND * 512))

    import ml_dtypes
    mask = np.asarray(attention_mask).astype(np.float32).reshape(1, T)
    kaug = ((mask - 1.0) * NEG_BIG).astype(f16)
    qi = np.arange(128)
    trilb = np.where(qi[None, :] >= qi[:, None], 1.0,
                     0.0).astype(ml_dtypes.bfloat16)   # keep[k,q]: q >= k
    identf = np.eye(128, dtype=np.float32)

    common = {
        **xh, "xhr": xhr, "kaug": kaug, "trilb": trilb,
        "identf": identf,
    }

    def pack_w(Wm, sl):
        wc = np.asarray(Wm, np.float32)[sl, :].astype(f16)  # [128e, D]
        return wc.reshape(128, ND, 128).transpose(2, 1, 0)   # [p, d, e]

    in_maps = []
    for c in range(N_CORES):
        sl = slice(c * E, (c + 1) * E)
        wq = pack_w(W_Q, sl)
        wk = pack_w(W_K, sl)
        wv = pack_w(W_V, sl)
        in_maps.append({
            **common,
            "wqk": np.ascontiguousarray(
                np.concatenate([wq, wk], axis=1).reshape(128, 2 * ND * 128)),
            "wv": np.ascontiguousarray(wv.reshape(128, ND * 128)),
        })
    return in_maps


def kernel(input, attention_mask, W_Q, W_K, W_V):
    from concourse.bass_utils import run_bass_kernel_spmd

    nc = _get_program()
    in_maps = _prep_inputs(input, attention_mask, W_Q, W_K, W_V)
    res = run_bass_kernel_spmd(nc, in_maps, list(range(N_CORES)))
    return np.concatenate([res.results[c]["out"] for c in range(N_CORES)],
                          axis=2)


# revision 61
# speedup vs baseline: 1.2647x; 1.0064x over previous
"""Trainium2 Bass kernel for sparse (causal, tanh-clamped) attention.

Problem: B=2, L=2048, D=1024, H=16 heads x 64 dim; S = QK^T/8;
S = 30*tanh(S); causal + attention_mask; softmax; out = attn @ V.

Sharding: 2 heads per core across 8 cores (tensor-parallel on heads).

The ACT (scalar) engine is the bottleneck: tanh+exp over every causal
score column is ~116us of irreducible element time; everything else is
arranged to keep ACT saturated and to minimize its per-instruction
overhead (~330ns per tanh/exp pair).

Design:
 - fp16 x/W/Q/K (precision for the score path: the exp amplifies score
   errors by d(30*tanh)/ds), bf16 probabilities/V (P can be ~e^-60;
   fp16 would flush it to 0 and NaN the softmax denominator). All
   matmuls cost 1 cycle/moving-row at 16-bit; DMA bytes are halved.
 - S^T[k, q] layout throughout; no P transpose (P^T feeds AV directly).
 - attention_mask folded into the score matmul via a 65th contraction
   row; causal diagonal masked by a tril MULTIPLY on the probabilities
   (an additive -big mask would leave e^-60 ghosts that dominate rows
   whose live probabilities are comparably small).
 - bounded logits: P = exp(30*tanh(s)-30), no running max; denominator
   comes free as a ones-column in the augmented V tile (po row 0).
 - V projected token-major directly (stationary=x chunk, moving=W_V
   chunk): no V transposes, no separate vaug copies.
 - merged-head spans (qw<=512): one [128, <=1024] PSUM strip holds both
   heads' score columns per k-tile, so a single tanh/exp pair covers
   both heads -- 82 ACT pairs total instead of 102.
 - AV accumulation per 128-column tile with stop at ki==j; bank drains
   to SBUF when retired, transpose/normalize chain deferred into the
   next span (psO-ring aliasing requires pts after po's last access).
   The last span drains per-128 with pt from the strip ring and
   per-tile stores on the sync queue for a ~4us tail.
 - software-pipelined emission: score(ki+1) prefetched ahead of AV/pump
   on the in-order PE queue; background work (projection chunks, V
   tiles, x loads, stores) flows through a budget-limited pump with
   explicit prerequisites, forced just-in-time at span boundaries with
   drains split across ACT/DVE when ACT would be idle anyway.
"""

import sys

if "/opt/trn_rl_repo" not in sys.path:
    sys.path.insert(0, "/opt/trn_rl_repo")

import numpy as np

B = 2
L = 2048
D = 1024
H = 16
DH = 64
N_CORES = 8
T = B * L            # 4096 tokens
E = 128              # per-core output features (2 heads)
ND = D // 128        # 8 contraction chunks
NEG_BIG = 6.0e4   # fp16-safe; still saturates tanh
TAU = 30.0

_CACHE = {}


def _build_program():
    import concourse.bacc as bacc
    import concourse.tile as tile
    from concourse import mybir
    from collections import deque

    F32 = mybir.dt.float32
    F16 = mybir.dt.float16
    BF16 = mybir.dt.bfloat16
    AF = mybir.ActivationFunctionType

    nc = bacc.Bacc("TRN2", target_bir_lowering=False, debug=False,
                   num_devices=N_CORES)

    # --- DRAM tensors -----------------------------------------------------
    # x host-packed: group0 of each batch split in two 256-token pieces
    # laid out [p, (d, t)], remaining groups [p, (g, d, t512)].
    xh0a_d = [nc.dram_tensor(f"xh0a{b}", [128, ND * 256], F16,
                             kind="ExternalInput") for b in range(B)]
    xh0b_d = [nc.dram_tensor(f"xh0b{b}", [128, ND * 256], F16,
                             kind="ExternalInput") for b in range(B)]
    xhr_d = nc.dram_tensor("xhr", [128, 6 * ND * 512], F16,
                           kind="ExternalInput")
    wqk_d = nc.dram_tensor("wqk", [128, 2 * ND * 128], F16,
                           kind="ExternalInput")
    wv_d = nc.dram_tensor("wv", [128, ND * 128], F16,
                          kind="ExternalInput")
    kaug_d = nc.dram_tensor("kaug", [1, T], F16, kind="ExternalInput")
    trilb_d = nc.dram_tensor("trilb", [128, 128], BF16,
                             kind="ExternalInput")
    identf_d = nc.dram_tensor("identf", [128, 128], F32,
                              kind="ExternalInput")
    out_d = nc.dram_tensor("out", [B, L, E], F32, kind="ExternalOutput")

    NJ = L // 128     # 16 column tiles per sequence

    with tile.TileContext(nc) as tc:
        with (
            tc.tile_pool(name="const", bufs=1) as constp,
            tc.tile_pool(name="weights", bufs=1) as wp,
            tc.tile_pool(name="qk", bufs=1) as qkp,
            tc.tile_pool(name="va", bufs=1) as vap,
            tc.tile_pool(name="x0", bufs=4) as x0p,
            tc.tile_pool(name="xg", bufs=6) as xgp,
            tc.tile_pool(name="prob", bufs=6) as ppp,
            tc.tile_pool(name="epi", bufs=6) as epip,
            tc.tile_pool(name="ostage", bufs=1) as ostagep,
            tc.tile_pool(name="strip", bufs=3, space="PSUM") as stripp,
            tc.tile_pool(name="psO", bufs=1, space="PSUM") as pop,
        ):
            # --- constants ----------------------------------------------
            trilb_t = constp.tile([128, 128], BF16, tag="trilb")
            identf_t = constp.tile([128, 128], F32, tag="identf")
            n30_t = constp.tile([128, 1], F32, tag="n30")
            wz_t = constp.tile([128, 128], F16, tag="wzero")
            nc.gpsimd.memset(wz_t[:], 0.0)

            wqk_t = wp.tile([128, 2 * ND * 128], F16, tag="wqk",
                            name="wqk")
            wv_t = wp.tile([128, ND * 128], F16, tag="wv", name="wv")

            QT = [[qkp.tile([65, L], F16, tag=f"qt{h}{b}",
                            name=f"qt{h}{b}")
                   for b in range(B)] for h in range(2)]
            KT = [[qkp.tile([65, L], F16, tag=f"kt{h}{b}",
                            name=f"kt{h}{b}")
                   for b in range(B)] for h in range(2)]
            # va[b][ki]: [128 tokens, 131]: col0=ones, 1:65=h0 feats,
            # col65=ones, 66:130=h1 feats
            VA = [[vap.tile([128, 131], BF16, tag=f"va{b}_{k}",
                            name=f"va{b}_{k}") for k in range(NJ)]
                  for b in range(B)]
            OST = [[ostagep.tile([128, 512], F32, tag=f"os{b}_{k}",
                                 name=f"os{b}_{k}") for k in range(4)]
                   for b in range(B)]

            # aug rows for batch 0 first: the first scores read QT
            # row 64, and these [1, 2048] memsets cost ~1.8us each on
            # the serial Pool queue
            for h in range(2):
                nc.gpsimd.memset(QT[h][0][64:65, :], 1.0)
            nc.gpsimd.memset(n30_t[:], -TAU)
            # ones columns of VA (written once; Pool is idle)
            for b in range(B):
                for k in range(NJ):
                    nc.gpsimd.memset(VA[b][k][:, 0:1], 1.0)
                    nc.gpsimd.memset(VA[b][k][:, 65:66], 1.0)
            for h in range(2):
                nc.gpsimd.memset(QT[h][1][64:65, :], 1.0)

            # --- initial DMAs. The startup chain is HWDGE-bound
            # (~625ns per dma_start on a serial device), so the minimum
            # number of transfers gates the first projection.
            x0_tiles = {}   # (b, half) -> tile
            x0_tiles[(0, 0)] = x0p.tile([128, ND * 256], F16, tag="x0",
                                        name="x00")
            nc.sync.dma_start(x0_tiles[(0, 0)][:], xh0a_d[0].ap()[:])
            nc.sync.dma_start(wqk_t[:, 0:1024], wqk_d.ap()[:, 0:1024])
            nc.sync.dma_start(wqk_t[:, 1024:2048],
                              wqk_d.ap()[:, 1024:2048])
            for h in range(2):
                nc.sync.dma_start(KT[h][0][64:65, :], kaug_d.ap()[0:1, 0:L])
            x0_tiles[(0, 1)] = x0p.tile([128, ND * 256], F16, tag="x0",
                                        name="x01")
            nc.sync.dma_start(x0_tiles[(0, 1)][:], xh0b_d[0].ap()[:])
            nc.sync.dma_start(trilb_t[:], trilb_d.ap()[:])
            nc.sync.dma_start(wv_t[:], wv_d.ap()[:])
            nc.sync.dma_start(identf_t[:], identf_d.ap()[:])

            # PE warmup: ramp the p-state and keep PE busy until the
            # first projection inputs land (a gap resets the ramp).
            wm = stripp.tile([128, 1024], F32, tag="strip",
                             name="warm")
            for _ in range(20):
                nc.tensor.matmul(wm[:, 0:128], wz_t[:], wz_t[:],
                                 start=True, stop=True)

            xg_tiles = {}   # group g (1..7) -> tile

            def load_group(g):
                t = xgp.tile([128, ND * 512], F16, tag="xg",
                             name=f"xg{g}")
                idx = g - 1 if g <= 3 else g - 2   # xhr: groups 1,2,3,5,6,7
                nc.sync.dma_start(
                    t[:], xhr_d.ap()[:, idx * 4096:(idx + 1) * 4096])
                xg_tiles[g] = t

            def load_aug_b1():
                for h in range(2):
                    nc.sync.dma_start(KT[h][1][64:65, :],
                                      kaug_d.ap()[0:1, L:2 * L])

            def x_slice(b, t0, width):
                """SBUF source tile for tokens [t0, t0+width) of batch b:
                returns (tile, per-d stride, column base)."""
                tglob = b * L + t0
                g = tglob // 512
                if g in (0, 4):   # each batch's first group: split tiles
                    half = (tglob % 512) // 256
                    return x0_tiles[(b, half)], 256, tglob % 256
                return xg_tiles[g], 512, tglob % 512

            def load_b1_first():
                for half in range(2):
                    t = x0p.tile([128, ND * 256], F16, tag="x0",
                                 name=f"x1{half}")
                    nc.sync.dma_start(t[:], (xh0a_d[1] if half == 0
                                             else xh0b_d[1]).ap()[:])
                    x0_tiles[(1, half)] = t

            # --- projection emitters ------------------------------------
            def qk_chunk_mm(b, c, dlo, dhi, pj):
                """QK projection matmuls for 256-token chunk c of batch
                b, d-chunks [dlo, dhi)."""
                t0 = 256 * c
                xt, tw, base = x_slice(b, t0, 256)
                for d in range(dlo, dhi):
                    xs = xt[:, d * tw + base:d * tw + base + 256]
                    nc.tensor.matmul(
                        pj[:, 0:256], wqk_t[:, d * 128:(d + 1) * 128],
                        xs, start=(d == 0), stop=(d == ND - 1))
                    nc.tensor.matmul(
                        pj[:, 256:512],
                        wqk_t[:, ND * 128 + d * 128:ND * 128 + (d + 1) * 128],
                        xs, start=False, stop=(d == ND - 1))

            def qk_chunk_drain(b, c, part, pj, on_act=False, hs=(0, 1)):
                """Drain chunk c: part 0 = Q, part 1 = K; heads hs."""
                t0 = 256 * c
                dst = QT if part == 0 else KT
                for h in hs:
                    src = pj[h * 64:(h + 1) * 64,
                             part * 256:part * 256 + 256]
                    d = dst[h][b][0:64, t0:t0 + 256]
                    if on_act:
                        nc.scalar.activation(d, src, AF.Identity)
                    else:
                        nc.vector.tensor_copy(d, src)

            def v_tile(b, ki):
                """V projection for token-tile ki of batch b:
                token-major via stationary-x matmul; drains into VA."""
                t0 = 128 * ki
                xt, tw, base = x_slice(b, t0, 128)
                pv = stripp.tile([128, 1024], F32, tag="strip",
                                 name="pv")
                for d in range(ND):
                    nc.tensor.matmul(
                        pv[:, 0:128],
                        xt[:, d * tw + base:d * tw + base + 128],
                        wv_t[:, d * 128:(d + 1) * 128],
                        start=(d == 0), stop=(d == ND - 1))
                nc.vector.tensor_copy(VA[b][ki][:, 1:65], pv[:, 0:64])
                nc.vector.tensor_copy(VA[b][ki][:, 66:130], pv[:, 64:128])

            # --- background queue ---------------------------------------
            bg = deque()
            emitted = set()
            by_key = {}

            def bg_add(key, pe, dve, fn, requires=()):
                it = {"key": key, "pe": pe, "dve": dve, "fn": fn,
                      "req": tuple(requires)}
                bg.append(it)
                by_key[key] = it

            def bg_run(item):
                if item["key"] in emitted:
                    return
                for r in item["req"]:
                    if r not in emitted and r in by_key:
                        bg_run(by_key[r])
                emitted.add(item["key"])
                item["fn"]()

            def pump(pe_budget, dve_budget):
                # scan past blocked items (requirements make out-of-order
                # emission safe); bounded lookahead keeps need-order bias.
                # At most one PSUM-using (PE) quantum per call: a second
                # would cycle the strip ring into a head-of-line stall.
                scanned = 0
                pe_quanta = 0
                i = 0
                while i < len(bg) and scanned < 12:
                    it = bg[i]
                    if it["key"] in emitted:
                        del bg[i]
                        continue
                    scanned += 1
                    fits = (it["pe"] <= pe_budget
                            and it["dve"] <= dve_budget
                            and (it["pe"] == 0 or pe_quanta < 1))
                    if fits:
                        del bg[i]
                        bg_run(it)
                        pe_budget -= it["pe"]
                        dve_budget -= it["dve"]
                        if it["pe"] > 0:
                            pe_quanta += 1
                    else:
                        i += 1

            def force(pred):
                # emit exactly the matching items (plus their declared
                # prerequisites), leaving unrelated queued work in place
                for it in list(bg):
                    if it["key"] not in emitted and pred(it["key"]):
                        bg_run(it)
                while bg and bg[0]["key"] in emitted:
                    bg.popleft()

            def x_req(b, t0):
                g = (b * L + t0) // 512
                if g == 0:
                    return ()
                if g == 4:
                    return (("x1a", 0),)
                return (("xg", g),)

            drain_fns = {}

            def queue_qk(b, c, act_drain=False):
                shared = {}
                req = x_req(b, 256 * c)

                def mk_mm(dlo, dhi):
                    def fn():
                        if "pj" not in shared:
                            shared["pj"] = stripp.tile([128, 1024], F32,
                                                       tag="strip",
                                                       name="pj")
                        qk_chunk_mm(b, c, dlo, dhi, shared["pj"])
                    return fn

                def mk_drain(part, h):
                    def fn(on_act=False):
                        qk_chunk_drain(b, c, part, shared["pj"],
                                       on_act=on_act, hs=(h,))
                    return fn
                for q in range(4):
                    bg_add(("qkm", b, c, q), 440, 0,
                           mk_mm(q * 2, q * 2 + 2),
                           req if q == 0 else (("qkm", b, c, q - 1),))
                for part in range(2):
                    for h in range(2):
                        fn = mk_drain(part, h)
                        drain_fns[(b, c, part, h)] = fn
                        bg_add(("qkd", b, c, part, h), 0, 400, fn,
                               (("qkm", b, c, 3),))

            def queue_v(b, ki):
                bg_add(("v", b, ki), 430, 390,
                       lambda b=b, ki=ki: v_tile(b, ki),
                       x_req(b, 128 * ki))

            def queue_dma(key, fn, requires=()):
                bg_add(key, 0, 0, fn, requires)

            # --- attention ----------------------------------------------
            def queue_store(b, blk):
                def fn(b=b, blk=blk):
                    nc.gpsimd.dma_start(
                        out_d.ap()[b, blk * 512:(blk + 1) * 512, :]
                        .rearrange("(j p) e -> p j e", p=128),
                        OST[b][blk][:].rearrange("p (j e) -> p j e", j=4))
                queue_dma(("store", b, blk), fn)

            def normalize_tile(b, h, j, dst):
                """dst: transposed [128, 65] PSUM view (col0 = denom)."""
                rec = epip.tile([128, 1], F32, tag="rec", name="rec")
                nc.vector.reciprocal(rec[:], dst[0:128, 0:1])
                blk, j_in = j // 4, j % 4
                nc.vector.tensor_scalar_mul(
                    OST[b][blk][:, j_in * 128 + h * 64:
                                j_in * 128 + h * 64 + 64],
                    dst[0:128, 1:65], rec[:])
                if h == 1:
                    if b == 1 and blk == 3:
                        # final block: per-tile stores on the sync queue
                        # (HWDGE; the SWDGE gen on Pool would serialize
                        # the last three stores at ~1us each)
                        def fn(b=b, j=j, j_in=j_in, blk=blk):
                            nc.sync.dma_start(
                                out_d.ap()[b, j * 128:(j + 1) * 128, :],
                                OST[b][blk][:, j_in * 128:
                                            (j_in + 1) * 128])
                        queue_dma(("store", b, blk, j), fn)
                    elif j % 4 == 3:
                        queue_store(b, blk)

            def attention_span(b, qlo, qw, last_span=False,
                               carry=None):
                """Merged-head span: one [128, <=1024] strip holds both
                heads' score columns for each k-tile; a single tanh/exp
                pair covers them (halves the ACT instruction count).
                qw <= 512. Head 1's block sits at offset `h1b`:
                packed at w when 2w <= 512, else at 512 (bank B)."""
                jlo, jhi = qlo // 128, (qlo + qw) // 128
                state = {0: None, 1: None}
                prev = None
                bank_ots = {}   # h -> ot tile for the span's bank

                def get_po(h):
                    if state[h] is None:
                        state[h] = pop.tile([65, 512], F32,
                                            tag=f"po{h}", name=f"po{h}")
                    return state[h]

                sc = {}

                def emit_score(ki):
                    q0 = max(qlo, ki * 128)
                    w = qlo + qw - q0
                    h1b = w if 2 * w <= 512 else 512
                    strip = stripp.tile([128, 1024], F32, tag="strip",
                                        name="strip")
                    for h in range(2):
                        nc.tensor.matmul(
                            strip[:, h * h1b:h * h1b + w],
                            KT[h][b][:, ki * 128:ki * 128 + 128],
                            QT[h][b][:, q0:q0 + w],
                            start=True, stop=True)
                    sc[ki] = (strip, q0, w, h1b)

                def emit_av(ki, pp, q0, w, h1b):
                    for h in range(2):
                        po = get_po(h)
                        for j in range(max(ki, jlo), jhi):
                            cl = h * h1b + j * 128 - q0
                            nc.tensor.matmul(
                                po[:, j * 128 - qlo:(j + 1) * 128 - qlo],
                                VA[b][ki][:, h * 65:h * 65 + 65],
                                pp[:, cl:cl + 128],
                                start=(ki == 0 and j == jlo),
                                stop=(ki == j))
                            if ki != j:
                                continue
                            if last_span and j >= jhi - 4:
                                ot = epip.tile([65, 512], F32,
                                               tag="ot", name="ot")
                                nc.vector.tensor_copy(
                                    ot[0:65, 0:128],
                                    po[:, j * 128 - qlo:
                                        (j + 1) * 128 - qlo])
                                pt = stripp.tile([128, 1024], F32,
                                                 tag="strip", name="pt")
                                nc.tensor.transpose(
                                    pt[0:128, 0:65], ot[0:65, 0:128],
                                    identf_t[0:65, 0:65])
                                normalize_tile(b, h, j,
                                               pt[0:128, 0:65])
                            elif j == jhi - 1:
                                nb = jhi - jlo
                                ot = epip.tile([65, 512], F32,
                                               tag="ot", name="ot")
                                nc.vector.tensor_copy(
                                    ot[0:65, 0:nb * 128],
                                    po[:, 0:nb * 128])
                                bank_ots[h] = (ot, jlo, nb)

                emit_score(0)
                for ki in range(jhi):
                    if ki + 1 < jhi:
                        emit_score(ki + 1)
                    strip, q0, w, h1b = sc.pop(ki)
                    diag = (q0 == ki * 128)
                    tot = h1b + w
                    pp = ppp.tile([128, 1024], BF16, tag="pp",
                                  name="pp")
                    if w < h1b:
                        # gapped layout (h1 at offset 512): process both
                        # blocks with one 3D AP, skipping the hole
                        sv = strip[:].rearrange("p (g c) -> p g c",
                                                g=2)[:, :, 0:w]
                        pv = pp[:].rearrange("p (g c) -> p g c",
                                             g=2)[:, :, 0:w]
                        nc.scalar.activation(sv, sv, AF.Tanh,
                                             scale=0.125)
                        nc.scalar.activation(pv, sv, AF.Exp,
                                             bias=n30_t[:], scale=TAU)
                    else:
                        nc.scalar.activation(strip[:, 0:tot],
                                             strip[:, 0:tot],
                                             AF.Tanh, scale=0.125)
                        nc.scalar.activation(pp[:, 0:tot],
                                             strip[:, 0:tot],
                                             AF.Exp, bias=n30_t[:],
                                             scale=TAU)
                    if diag:
                        # exact-zero the below-diagonal ghosts
                        nc.vector.tensor_mul(pp[:, 0:128],
                                             pp[:, 0:128], trilb_t[:])
                        nc.vector.tensor_mul(
                            pp[:, h1b:h1b + 128],
                            pp[:, h1b:h1b + 128], trilb_t[:])
                    if carry is not None:
                        carry()
                        carry = None
                    if prev is not None:
                        if not ("v", b, prev[0]) in emitted:
                            force(lambda k, kk=prev[0]:
                                  k == ("v", b, kk))
                        emit_av(*prev)
                    act_ns = 1.67 * tot + 330
                    pe_ns = 0.43 * tot + (120 if diag else 0) + \
                        0.86 * 128 * max(0, jhi - max(ki - 1, jlo)) + 100
                    pump(max(0.0, act_ns - pe_ns - 100),
                         max(0.0, act_ns - 550))
                    prev = (ki, pp, q0, w, h1b)
                if prev is not None:
                    if not ("v", b, prev[0]) in emitted:
                        force(lambda k, kk=prev[0]: k == ("v", b, kk))
                    emit_av(*prev)

                def finish():
                    for h in sorted(bank_ots):
                        ot, jb, nb = bank_ots[h]
                        for jj in range(nb):
                            pt = pop.tile([128, 65], F32, tag="po0",
                                          name="pt")
                            nc.tensor.transpose(
                                pt[:], ot[0:65, jj * 128:(jj + 1) * 128],
                                identf_t[0:65, 0:65])
                            normalize_tile(b, h, jb + jj, pt[:])
                return finish

            # --- orchestration ------------------------------------------
            # Spans alternate heads: h1 re-uses h0's projections, so the
            # ACT work per projection deadline doubles and background
            # projection quanta fit inside the span slack.
            # prologue: QK chunk 0 + V tiles 0-1 of b0 (drains split
            # ACT/DVE to shorten the startup chain)
            pj0 = stripp.tile([128, 1024], F32, tag="strip",
                              name="pj0")
            # all Q matmuls before all K: the wk DMA lands after wq, and
            # interleaving would head-of-line block Q matmuls behind K(d0)
            xt0, tw0, base0 = x_slice(0, 0, 256)
            for d in range(ND):
                nc.tensor.matmul(
                    pj0[:, 0:256], wqk_t[:, d * 128:(d + 1) * 128],
                    xt0[:, d * tw0 + base0:d * tw0 + base0 + 256],
                    start=(d == 0), stop=(d == ND - 1))
            for d in range(ND):
                nc.tensor.matmul(
                    pj0[:, 256:512],
                    wqk_t[:, ND * 128 + d * 128:ND * 128 + (d + 1) * 128],
                    xt0[:, d * tw0 + base0:d * tw0 + base0 + 256],
                    start=False, stop=(d == ND - 1))
            qk_chunk_drain(0, 0, 0, pj0, on_act=True, hs=(0,))
            qk_chunk_drain(0, 0, 1, pj0, on_act=True, hs=(0,))
            qk_chunk_drain(0, 0, 0, pj0, on_act=False, hs=(1,))
            qk_chunk_drain(0, 0, 1, pj0, on_act=False, hs=(1,))
            for ki in (0, 1):
                queue_v(0, ki)

            # chunk 1 matmuls inline during the ACT-idle startup;
            # its drains go to the background queue
            pj1 = stripp.tile([128, 1024], F32, tag="strip",
                              name="pj1")
            qk_chunk_mm(0, 1, 0, ND, pj1)
            for part in range(2):
                for h in range(2):
                    def c1drain(on_act=False, p=part, hh=h):
                        qk_chunk_drain(0, 1, p, pj1, on_act=on_act,
                                       hs=(hh,))
                    drain_fns[(0, 1, part, h)] = c1drain
                    bg_add(("qkd", 0, 1, part, h), 0, 400, c1drain)
            for q in range(4):
                emitted.add(("qkm", 0, 1, q))


            for g in (1, 2, 3):
                queue_dma(("xg", g), lambda g=g: load_group(g))
            for c in (2, 3):
                queue_qk(0, c)
            for ki in (2, 3, 4, 5):
                queue_v(0, ki)
            for c in (4, 5):
                queue_qk(0, c)
            for ki in (6, 7, 8, 9):
                queue_v(0, ki)
            for c in (6, 7):
                queue_qk(0, c)
            for ki in range(10, NJ):
                queue_v(0, ki)

            def need_qk(b, cs):
                force(lambda k: k[0] == "qkm" and k[1] == b
                      and k[2] in cs)
                # bulk-forced drains: ACT is idle while it waits for
                # these, so alternate them between ACT and DVE
                n = 0
                for c in cs:
                    for part in range(2):
                        for h in range(2):
                            key = ("qkd", b, c, part, h)
                            if key in emitted or key not in by_key:
                                continue
                            emitted.add(key)
                            drain_fns[(b, c, part, h)](on_act=n % 2 == 0)
                            n += 1

            def tk(*keys):
                return deque(keys)

            def chunk_keys(b, *cs):
                out = []
                for c in cs:
                    out += [("qkm", b, c, q) for q in range(4)]
                    out += [("qkd", b, c, p, hh)
                            for p in range(2) for hh in range(2)]
                return deque(out)

            # batch 0 (merged-head spans, qw <= 512)
            cr = attention_span(0, 0, 256)
            need_qk(0, (1,))
            cr = attention_span(0, 256, 256, carry=cr)
            need_qk(0, (2, 3))
            cr = attention_span(0, 512, 512, carry=cr)
            # queue b1 inputs + b1 projection work mid-flight
            queue_dma(("x1a", 0), load_b1_first)
            queue_dma(("aug1", 0), load_aug_b1)
            for g in (5, 6, 7):
                queue_dma(("xg", g), lambda g=g: load_group(g))
            for c in (0, 1, 2, 3):
                queue_qk(1, c)
            for ki in (0, 1, 2, 3):
                queue_v(1, ki)
            for c in (4, 5, 6, 7):
                queue_qk(1, c)
            for ki in range(4, NJ):
                queue_v(1, ki)
            need_qk(0, (4, 5))
            cr = attention_span(0, 1024, 512, carry=cr)
            need_qk(0, (6, 7))
            cr = attention_span(0, 1536, 512, carry=cr)

            # batch 1
            force(lambda k: k[0] in ("x1a", "aug1"))
            need_qk(1, (0, 1))
            cr = attention_span(1, 0, 512, carry=cr)
            need_qk(1, (2, 3))
            cr = attention_span(1, 512, 512, carry=cr)
            need_qk(1, (4, 5))
            cr = attention_span(1, 1024, 512, carry=cr)
            need_qk(1, (6, 7))
            cr = attention_span(1, 1536, 512, last_span=True, carry=cr)
            cr()
            force(lambda k: True)

    nc.compile()
    return nc


def _get_program():
    if "nc" not in _CACHE:
        _CACHE["nc"] = _build_program()
    return _CACHE["nc"]


def _prep_inputs(input, attention_mask, W_Q, W_K, W_V):
    f16 = np.float16

    x = np.asarray(input, dtype=np.float32).reshape(T, D)
    xT = np.ascontiguousarray(x.T).astype(f16)             # [D, T]
    # [d, p, g, tloc]
    xr = xT.reshape(ND, 128, B * 4, 512).transpose(1, 2, 0, 3)
    # xr: [p, g, d, tloc]
    xh = {}
    for b in range(B):
        g0 = xr[:, b * 4]                                    # [p, d, 512]
        xh[f"xh0a{b}"] = np.ascontiguousarray(
            g0[:, :, 0:256].reshape(128, ND * 256))
        xh[f"xh0b{b}"] = np.ascontiguousarray(
            g0[:, :, 256:512].reshape(128, ND * 256))
    rest = np.concatenate([xr[:, g] for g in (1, 2, 3, 5, 6, 7)],
                          axis=1)                            # [p, 6*d, 512]
    xhr = np.ascontiguousarray(rest.reshape(128, 6 * ND * 512))

    import ml_dtypes
    mask = np.asarray(attention_mask).astype(np.float32).reshape(1, T)
    kaug = ((mask - 1.0) * NEG_BIG).astype(f16)
    qi = np.arange(128)
    trilb = np.where(qi[None, :] >= qi[:, None], 1.0,
                     0.0).astype(ml_dtypes.bfloat16)   # keep[k,q]: q >= k
    identf = np.eye(128, dtype=np.float32)

    common = {
        **xh, "xhr": xhr, "kaug": kaug, "trilb": trilb,
        "identf": identf,
    }

    def pack_w(Wm, sl):
        wc = np.asarray(Wm, np.float32)[sl, :].astype(f16)  # [128e, D]
        return wc.reshape(128, ND, 128).transpose(2, 1, 0)   # [p, d, e]

    in_maps = []
    for c in range(N_CORES):
        sl = slice(c * E, (c + 1) * E)
        wq = pack_w(W_Q, sl)
        wk = pack_w(W_K, sl)
        wv = pack_w(W_V, sl)
        in_maps.append({
            **common,
            "wqk": np.ascontiguousarray(
                np.concatenate([wq, wk], axis=1).reshape(128, 2 * ND * 128)),
            "wv": np.ascontiguousarray(wv.reshape(128, ND * 128)),
        })
    return in_maps


def kernel(input, attention_mask, W_Q, W_K, W_V):
    from concourse.bass_utils import run_bass_kernel_spmd

    nc = _get_program()
    in_maps = _prep_inputs(input, attention_mask, W_Q, W_K, W_V)
    res = run_bass_kernel_spmd(nc, in_maps, list(range(N_CORES)))
    return np.concatenate([res.results[c]["out"] for c in range(N_CORES)],
                          axis=2)


# revision 64
# speedup vs baseline: 1.2684x; 1.0029x over previous
"""Trainium2 Bass kernel for sparse (causal, tanh-clamped) attention.

Problem: B=2, L=2048, D=1024, H=16 heads x 64 dim; S = QK^T/8;
S = 30*tanh(S); causal + attention_mask; softmax; out = attn @ V.

Sharding: 2 heads per core across 8 cores (tensor-parallel on heads).

The ACT (scalar) engine is the bottleneck: tanh+exp over every causal
score column is ~116us of irreducible element time; everything else is
arranged to keep ACT saturated and to minimize its per-instruction
overhead (~330ns per tanh/exp pair).

Design:
 - fp16 x/W/Q/K (precision for the score path: the exp amplifies score
   errors by d(30*tanh)/ds), bf16 probabilities/V (P can be ~e^-60;
   fp16 would flush it to 0 and NaN the softmax denominator). All
   matmuls cost 1 cycle/moving-row at 16-bit; DMA bytes are halved.
 - S^T[k, q] layout throughout; no P transpose (P^T feeds AV directly).
 - attention_mask folded into the score matmul via a 65th contraction
   row; causal diagonal masked by a tril MULTIPLY on the probabilities
   (an additive -big mask would leave e^-60 ghosts that dominate rows
   whose live probabilities are comparably small).
 - bounded logits: P = exp(30*tanh(s)-30), no running max; denominator
   comes free as a ones-column in the augmented V tile (po row 0).
 - V projected token-major directly (stationary=x chunk, moving=W_V
   chunk): no V transposes, no separate vaug copies.
 - merged-head spans (qw<=512): one [128, <=1024] PSUM strip holds both
   heads' score columns per k-tile, so a single tanh/exp pair covers
   both heads -- 82 ACT pairs total instead of 102.
 - AV accumulation per 128-column tile with stop at ki==j; bank drains
   to SBUF when retired, transpose/normalize chain deferred into the
   next span (psO-ring aliasing requires pts after po's last access).
   The last span drains per-128 with pt from the strip ring and
   per-tile stores on the sync queue for a ~4us tail.
 - software-pipelined emission: score(ki+1) prefetched ahead of AV/pump
   on the in-order PE queue; background work (projection chunks, V
   tiles, x loads, stores) flows through a budget-limited pump with
   explicit prerequisites, forced just-in-time at span boundaries with
   drains split across ACT/DVE when ACT would be idle anyway.
"""

import sys

if "/opt/trn_rl_repo" not in sys.path:
    sys.path.insert(0, "/opt/trn_rl_repo")

import numpy as np

B = 2
L = 2048
D = 1024
H = 16
DH = 64
N_CORES = 8
T = B * L            # 4096 tokens
E = 128              # per-core output features (2 heads)
ND = D // 128        # 8 contraction chunks
NEG_BIG = 6.0e4   # fp16-safe; still saturates tanh
TAU = 30.0

_CACHE = {}


def _build_program():
    import concourse.bacc as bacc
    import concourse.tile as tile
    from concourse import mybir
    from collections import deque

    F32 = mybir.dt.float32
    F16 = mybir.dt.float16
    BF16 = mybir.dt.bfloat16
    AF = mybir.ActivationFunctionType

    nc = bacc.Bacc("TRN2", target_bir_lowering=False, debug=False,
                   num_devices=N_CORES)

    # --- DRAM tensors -----------------------------------------------------
    # x host-packed: group0 of each batch split in two 256-token pieces
    # laid out [p, (d, t)], remaining groups [p, (g, d, t512)].
    xh0a_d = [nc.dram_tensor(f"xh0a{b}", [128, ND * 256], F16,
                             kind="ExternalInput") for b in range(B)]
    xh0b_d = [nc.dram_tensor(f"xh0b{b}", [128, ND * 256], F16,
                             kind="ExternalInput") for b in range(B)]
    xhr_d = nc.dram_tensor("xhr", [128, 6 * ND * 512], F16,
                           kind="ExternalInput")
    wqk_d = nc.dram_tensor("wqk", [128, 2 * ND * 128], F16,
                           kind="ExternalInput")
    wv_d = nc.dram_tensor("wv", [128, ND * 128], F16,
                          kind="ExternalInput")
    kaug_d = nc.dram_tensor("kaug", [1, T], F16, kind="ExternalInput")
    trilb_d = nc.dram_tensor("trilb", [128, 128], BF16,
                             kind="ExternalInput")
    identf_d = nc.dram_tensor("identf", [128, 128], F32,
                              kind="ExternalInput")
    out_d = nc.dram_tensor("out", [B, L, E], F32, kind="ExternalOutput")

    NJ = L // 128     # 16 column tiles per sequence

    with tile.TileContext(nc) as tc:
        with (
            tc.tile_pool(name="const", bufs=1) as constp,
            tc.tile_pool(name="weights", bufs=1) as wp,
            tc.tile_pool(name="qk", bufs=1) as qkp,
            tc.tile_pool(name="va", bufs=1) as vap,
            tc.tile_pool(name="x0", bufs=4) as x0p,
            tc.tile_pool(name="xg", bufs=6) as xgp,
            tc.tile_pool(name="prob", bufs=6) as ppp,
            tc.tile_pool(name="epi", bufs=6) as epip,
            tc.tile_pool(name="ostage", bufs=1) as ostagep,
            tc.tile_pool(name="strip", bufs=3, space="PSUM") as stripp,
            tc.tile_pool(name="psO", bufs=1, space="PSUM") as pop,
        ):
            # --- constants ----------------------------------------------
            trilb_t = constp.tile([128, 128], BF16, tag="trilb")
            identf_t = constp.tile([128, 128], F32, tag="identf")
            n30_t = constp.tile([128, 1], F32, tag="n30")
            wz_t = constp.tile([128, 128], F16, tag="wzero")
            nc.gpsimd.memset(wz_t[:], 0.0)

            wqk_t = wp.tile([128, 2 * ND * 128], F16, tag="wqk",
                            name="wqk")
            wv_t = wp.tile([128, ND * 128], F16, tag="wv", name="wv")

            QT = [[qkp.tile([65, L], F16, tag=f"qt{h}{b}",
                            name=f"qt{h}{b}")
                   for b in range(B)] for h in range(2)]
            KT = [[qkp.tile([65, L], F16, tag=f"kt{h}{b}",
                            name=f"kt{h}{b}")
                   for b in range(B)] for h in range(2)]
            # va[b][ki]: [128 tokens, 131]: col0=ones, 1:65=h0 feats,
            # col65=ones, 66:130=h1 feats
            VA = [[vap.tile([128, 131], BF16, tag=f"va{b}_{k}",
                            name=f"va{b}_{k}") for k in range(NJ)]
                  for b in range(B)]
            OST = [[ostagep.tile([128, 512], F32, tag=f"os{b}_{k}",
                                 name=f"os{b}_{k}") for k in range(4)]
                   for b in range(B)]

            # aug rows for batch 0 first: the first scores read QT
            # row 64, and these [1, 2048] memsets cost ~1.8us each on
            # the serial Pool queue
            for h in range(2):
                nc.gpsimd.memset(QT[h][0][64:65, :], 1.0)
            nc.gpsimd.memset(n30_t[:], -TAU)
            # ones columns of VA (written once; Pool is idle)
            for b in range(B):
                for k in range(NJ):
                    nc.gpsimd.memset(VA[b][k][:, 0:1], 1.0)
                    nc.gpsimd.memset(VA[b][k][:, 65:66], 1.0)
            for h in range(2):
                nc.gpsimd.memset(QT[h][1][64:65, :], 1.0)

            # --- initial DMAs. The startup chain is HWDGE-bound
            # (~625ns per dma_start on a serial device), so the minimum
            # number of transfers gates the first projection.
            x0_tiles = {}   # (b, half) -> tile
            x0_tiles[(0, 0)] = x0p.tile([128, ND * 256], F16, tag="x0",
                                        name="x00")
            nc.sync.dma_start(x0_tiles[(0, 0)][:], xh0a_d[0].ap()[:])
            nc.sync.dma_start(wqk_t[:, 0:1024], wqk_d.ap()[:, 0:1024])
            nc.sync.dma_start(wqk_t[:, 1024:2048],
                              wqk_d.ap()[:, 1024:2048])
            for h in range(2):
                nc.sync.dma_start(KT[h][0][64:65, :], kaug_d.ap()[0:1, 0:L])
            x0_tiles[(0, 1)] = x0p.tile([128, ND * 256], F16, tag="x0",
                                        name="x01")
            nc.sync.dma_start(x0_tiles[(0, 1)][:], xh0b_d[0].ap()[:])
            nc.sync.dma_start(trilb_t[:], trilb_d.ap()[:])
            nc.sync.dma_start(wv_t[:], wv_d.ap()[:])
            nc.sync.dma_start(identf_t[:], identf_d.ap()[:])

            # PE warmup: ramp the p-state and keep PE busy until the
            # first projection inputs land (a gap resets the ramp).
            wm = stripp.tile([128, 1024], F32, tag="strip",
                             name="warm")
            for _ in range(20):
                nc.tensor.matmul(wm[:, 0:128], wz_t[:], wz_t[:],
                                 start=True, stop=True)

            xg_tiles = {}   # group g (1..7) -> tile

            def load_group(g):
                t = xgp.tile([128, ND * 512], F16, tag="xg",
                             name=f"xg{g}")
                idx = g - 1 if g <= 3 else g - 2   # xhr: groups 1,2,3,5,6,7
                nc.sync.dma_start(
                    t[:], xhr_d.ap()[:, idx * 4096:(idx + 1) * 4096])
                xg_tiles[g] = t

            def load_aug_b1():
                for h in range(2):
                    nc.sync.dma_start(KT[h][1][64:65, :],
                                      kaug_d.ap()[0:1, L:2 * L])

            def x_slice(b, t0, width):
                """SBUF source tile for tokens [t0, t0+width) of batch b:
                returns (tile, per-d stride, column base)."""
                tglob = b * L + t0
                g = tglob // 512
                if g in (0, 4):   # each batch's first group: split tiles
                    half = (tglob % 512) // 256
                    return x0_tiles[(b, half)], 256, tglob % 256
                return xg_tiles[g], 512, tglob % 512

            def load_b1_first():
                for half in range(2):
                    t = x0p.tile([128, ND * 256], F16, tag="x0",
                                 name=f"x1{half}")
                    nc.sync.dma_start(t[:], (xh0a_d[1] if half == 0
                                             else xh0b_d[1]).ap()[:])
                    x0_tiles[(1, half)] = t

            # --- projection emitters ------------------------------------
            def qk_chunk_mm(b, c, dlo, dhi, pj):
                """QK projection matmuls for 256-token chunk c of batch
                b, d-chunks [dlo, dhi)."""
                t0 = 256 * c
                xt, tw, base = x_slice(b, t0, 256)
                for d in range(dlo, dhi):
                    xs = xt[:, d * tw + base:d * tw + base + 256]
                    nc.tensor.matmul(
                        pj[:, 0:256], wqk_t[:, d * 128:(d + 1) * 128],
                        xs, start=(d == 0), stop=(d == ND - 1))
                    nc.tensor.matmul(
                        pj[:, 256:512],
                        wqk_t[:, ND * 128 + d * 128:ND * 128 + (d + 1) * 128],
                        xs, start=False, stop=(d == ND - 1))

            def qk_chunk_drain(b, c, part, pj, on_act=False, hs=(0, 1)):
                """Drain chunk c: part 0 = Q, part 1 = K; heads hs."""
                t0 = 256 * c
                dst = QT if part == 0 else KT
                for h in hs:
                    src = pj[h * 64:(h + 1) * 64,
                             part * 256:part * 256 + 256]
                    d = dst[h][b][0:64, t0:t0 + 256]
                    if on_act:
                        nc.scalar.activation(d, src, AF.Identity)
                    else:
                        nc.vector.tensor_copy(d, src)

            def v_tile(b, ki):
                """V projection for token-tile ki of batch b:
                token-major via stationary-x matmul; drains into VA."""
                t0 = 128 * ki
                xt, tw, base = x_slice(b, t0, 128)
                pv = stripp.tile([128, 1024], F32, tag="strip",
                                 name="pv")
                for d in range(ND):
                    nc.tensor.matmul(
                        pv[:, 0:128],
                        xt[:, d * tw + base:d * tw + base + 128],
                        wv_t[:, d * 128:(d + 1) * 128],
                        start=(d == 0), stop=(d == ND - 1))
                nc.vector.tensor_copy(VA[b][ki][:, 1:65], pv[:, 0:64])
                nc.vector.tensor_copy(VA[b][ki][:, 66:130], pv[:, 64:128])

            # --- background queue ---------------------------------------
            bg = deque()
            emitted = set()
            by_key = {}

            def bg_add(key, pe, dve, fn, requires=()):
                it = {"key": key, "pe": pe, "dve": dve, "fn": fn,
                      "req": tuple(requires)}
                bg.append(it)
                by_key[key] = it

            def bg_run(item):
                if item["key"] in emitted:
                    return
                for r in item["req"]:
                    if r not in emitted and r in by_key:
                        bg_run(by_key[r])
                emitted.add(item["key"])
                item["fn"]()

            def pump(pe_budget, dve_budget):
                # scan past blocked items (requirements make out-of-order
                # emission safe); bounded lookahead keeps need-order bias.
                # At most one PSUM-using (PE) quantum per call: a second
                # would cycle the strip ring into a head-of-line stall.
                scanned = 0
                pe_quanta = 0
                i = 0
                while i < len(bg) and scanned < 12:
                    it = bg[i]
                    if it["key"] in emitted:
                        del bg[i]
                        continue
                    scanned += 1
                    fits = (it["pe"] <= pe_budget
                            and it["dve"] <= dve_budget
                            and (it["pe"] == 0 or pe_quanta < 1))
                    if fits:
                        del bg[i]
                        bg_run(it)
                        pe_budget -= it["pe"]
                        dve_budget -= it["dve"]
                        if it["pe"] > 0:
                            pe_quanta += 1
                    else:
                        i += 1

            def force(pred):
                # emit exactly the matching items (plus their declared
                # prerequisites), leaving unrelated queued work in place
                for it in list(bg):
                    if it["key"] not in emitted and pred(it["key"]):
                        bg_run(it)
                while bg and bg[0]["key"] in emitted:
                    bg.popleft()

            def x_req(b, t0):
                g = (b * L + t0) // 512
                if g == 0:
                    return ()
                if g == 4:
                    return (("x1a", 0),)
                return (("xg", g),)

            drain_fns = {}

            def queue_qk(b, c, act_drain=False):
                shared = {}
                req = x_req(b, 256 * c)

                def mk_mm(dlo, dhi):
                    def fn():
                        if "pj" not in shared:
                            shared["pj"] = stripp.tile([128, 1024], F32,
                                                       tag="strip",
                                                       name="pj")
                        qk_chunk_mm(b, c, dlo, dhi, shared["pj"])
                    return fn

                def mk_drain(part, h):
                    def fn(on_act=False):
                        qk_chunk_drain(b, c, part, shared["pj"],
                                       on_act=on_act, hs=(h,))
                    return fn
                for q in range(4):
                    bg_add(("qkm", b, c, q), 440, 0,
                           mk_mm(q * 2, q * 2 + 2),
                           req if q == 0 else (("qkm", b, c, q - 1),))
                for part in range(2):
                    for h in range(2):
                        fn = mk_drain(part, h)
                        drain_fns[(b, c, part, h)] = fn
                        bg_add(("qkd", b, c, part, h), 0, 400, fn,
                               (("qkm", b, c, 3),))

            def queue_v(b, ki):
                bg_add(("v", b, ki), 430, 390,
                       lambda b=b, ki=ki: v_tile(b, ki),
                       x_req(b, 128 * ki))

            def queue_dma(key, fn, requires=()):
                bg_add(key, 0, 0, fn, requires)

            # --- attention ----------------------------------------------
            def queue_store(b, blk):
                def fn(b=b, blk=blk):
                    nc.gpsimd.dma_start(
                        out_d.ap()[b, blk * 512:(blk + 1) * 512, :]
                        .rearrange("(j p) e -> p j e", p=128),
                        OST[b][blk][:].rearrange("p (j e) -> p j e", j=4))
                queue_dma(("store", b, blk), fn)

            def normalize_tile(b, h, j, dst):
                """dst: transposed [128, 65] PSUM view (col0 = denom)."""
                rec = epip.tile([128, 1], F32, tag="rec", name="rec")
                nc.vector.reciprocal(rec[:], dst[0:128, 0:1])
                blk, j_in = j // 4, j % 4
                nc.vector.tensor_scalar_mul(
                    OST[b][blk][:, j_in * 128 + h * 64:
                                j_in * 128 + h * 64 + 64],
                    dst[0:128, 1:65], rec[:])
                if h == 1:
                    if b == 1 and blk == 3:
                        # final block: per-tile stores on the sync queue
                        # (HWDGE; the SWDGE gen on Pool would serialize
                        # the last three stores at ~1us each)
                        def fn(b=b, j=j, j_in=j_in, blk=blk):
                            nc.sync.dma_start(
                                out_d.ap()[b, j * 128:(j + 1) * 128, :],
                                OST[b][blk][:, j_in * 128:
                                            (j_in + 1) * 128])
                        queue_dma(("store", b, blk, j), fn)
                    elif j % 4 == 3:
                        queue_store(b, blk)

            def attention_span(b, qlo, qw, last_span=False,
                               carry=None):
                """Merged-head span: one [128, <=1024] strip holds both
                heads' score columns for each k-tile; a single tanh/exp
                pair covers them (halves the ACT instruction count).
                qw <= 512. Head 1's block sits at offset `h1b`:
                packed at w when 2w <= 512, else at 512 (bank B)."""
                jlo, jhi = qlo // 128, (qlo + qw) // 128
                state = {0: None, 1: None}
                prev = None
                bank_ots = {}   # h -> ot tile for the span's bank

                def get_po(h):
                    if state[h] is None:
                        state[h] = pop.tile([65, 512], F32,
                                            tag=f"po{h}", name=f"po{h}")
                    return state[h]

                sc = {}

                def emit_score(ki):
                    q0 = max(qlo, ki * 128)
                    w = qlo + qw - q0
                    h1b = w if 2 * w <= 512 else 512
                    strip = stripp.tile([128, 1024], F32, tag="strip",
                                        name="strip")
                    for h in range(2):
                        nc.tensor.matmul(
                            strip[:, h * h1b:h * h1b + w],
                            KT[h][b][:, ki * 128:ki * 128 + 128],
                            QT[h][b][:, q0:q0 + w],
                            start=True, stop=True)
                    sc[ki] = (strip, q0, w, h1b)

                def emit_av(ki, pp, q0, w, h1b):
                    for h in range(2):
                        po = get_po(h)
                        for j in range(max(ki, jlo), jhi):
                            cl = h * h1b + j * 128 - q0
                            nc.tensor.matmul(
                                po[:, j * 128 - qlo:(j + 1) * 128 - qlo],
                                VA[b][ki][:, h * 65:h * 65 + 65],
                                pp[:, cl:cl + 128],
                                start=(ki == 0 and j == jlo),
                                stop=(ki == j))
                            if ki != j:
                                continue
                            if last_span and j >= jhi - 4:
                                ot = epip.tile([65, 512], F32,
                                               tag="ot", name="ot")
                                nc.vector.tensor_copy(
                                    ot[0:65, 0:128],
                                    po[:, j * 128 - qlo:
                                        (j + 1) * 128 - qlo])
                                pt = stripp.tile([128, 1024], F32,
                                                 tag="strip", name="pt")
                                nc.tensor.transpose(
                                    pt[0:128, 0:65], ot[0:65, 0:128],
                                    identf_t[0:65, 0:65])
                                normalize_tile(b, h, j,
                                               pt[0:128, 0:65])
                            elif j == jhi - 1:
                                nb = jhi - jlo
                                ot = epip.tile([65, 512], F32,
                                               tag="ot", name="ot")
                                nc.vector.tensor_copy(
                                    ot[0:65, 0:nb * 128],
                                    po[:, 0:nb * 128])
                                bank_ots[h] = (ot, jlo, nb)

                emit_score(0)
                for ki in range(jhi):
                    if ki + 1 < jhi:
                        emit_score(ki + 1)
                    strip, q0, w, h1b = sc.pop(ki)
                    diag = (q0 == ki * 128)
                    tot = h1b + w
                    pp = ppp.tile([128, 1024], BF16, tag="pp",
                                  name="pp")
                    if w < h1b:
                        # gapped layout (h1 at offset 512): process both
                        # blocks with one 3D AP, skipping the hole
                        sv = strip[:].rearrange("p (g c) -> p g c",
                                                g=2)[:, :, 0:w]
                        pv = pp[:].rearrange("p (g c) -> p g c",
                                             g=2)[:, :, 0:w]
                        nc.scalar.activation(sv, sv, AF.Tanh,
                                             scale=0.125)
                        nc.scalar.activation(pv, sv, AF.Exp,
                                             bias=n30_t[:], scale=TAU)
                    else:
                        nc.scalar.activation(strip[:, 0:tot],
                                             strip[:, 0:tot],
                                             AF.Tanh, scale=0.125)
                        nc.scalar.activation(pp[:, 0:tot],
                                             strip[:, 0:tot],
                                             AF.Exp, bias=n30_t[:],
                                             scale=TAU)
                    if diag:
                        # exact-zero the below-diagonal ghosts
                        nc.vector.tensor_mul(pp[:, 0:128],
                                             pp[:, 0:128], trilb_t[:])
                        nc.vector.tensor_mul(
                            pp[:, h1b:h1b + 128],
                            pp[:, h1b:h1b + 128], trilb_t[:])
                    if carry is not None:
                        carry()
                        carry = None
                    if prev is not None:
                        if not ("v", b, prev[0]) in emitted:
                            force(lambda k, kk=prev[0]:
                                  k == ("v", b, kk))
                        emit_av(*prev)
                    act_ns = 1.67 * tot + 330
                    pe_ns = 0.43 * tot + (120 if diag else 0) + \
                        0.86 * 128 * max(0, jhi - max(ki - 1, jlo)) + 100
                    pump(max(0.0, act_ns - pe_ns - 100),
                         max(0.0, act_ns - 550))
                    prev = (ki, pp, q0, w, h1b)
                if prev is not None:
                    if not ("v", b, prev[0]) in emitted:
                        force(lambda k, kk=prev[0]: k == ("v", b, kk))
                    emit_av(*prev)

                def finish():
                    for h in sorted(bank_ots):
                        ot, jb, nb = bank_ots[h]
                        for jj in range(nb):
                            pt = pop.tile([128, 65], F32, tag="po0",
                                          name="pt")
                            nc.tensor.transpose(
                                pt[:], ot[0:65, jj * 128:(jj + 1) * 128],
                                identf_t[0:65, 0:65])
                            normalize_tile(b, h, jb + jj, pt[:])
                return finish

            # --- orchestration ------------------------------------------
            # Spans alternate heads: h1 re-uses h0's projections, so the
            # ACT work per projection deadline doubles and background
            # projection quanta fit inside the span slack.
            # prologue: QK chunk 0 + V tiles 0-1 of b0 (drains split
            # ACT/DVE to shorten the startup chain)
            pj0 = stripp.tile([128, 1024], F32, tag="strip",
                              name="pj0")
            # all Q matmuls before all K: the wk DMA lands after wq, and
            # interleaving would head-of-line block Q matmuls behind K(d0)
            xt0, tw0, base0 = x_slice(0, 0, 256)
            for d in range(ND):
                nc.tensor.matmul(
                    pj0[:, 0:256], wqk_t[:, d * 128:(d + 1) * 128],
                    xt0[:, d * tw0 + base0:d * tw0 + base0 + 256],
                    start=(d == 0), stop=(d == ND - 1))
            for d in range(ND):
                nc.tensor.matmul(
                    pj0[:, 256:512],
                    wqk_t[:, ND * 128 + d * 128:ND * 128 + (d + 1) * 128],
                    xt0[:, d * tw0 + base0:d * tw0 + base0 + 256],
                    start=False, stop=(d == ND - 1))
            qk_chunk_drain(0, 0, 0, pj0, on_act=True, hs=(0,))
            qk_chunk_drain(0, 0, 1, pj0, on_act=True, hs=(0,))
            qk_chunk_drain(0, 0, 0, pj0, on_act=False, hs=(1,))
            qk_chunk_drain(0, 0, 1, pj0, on_act=False, hs=(1,))
            for ki in (0, 1):
                queue_v(0, ki)

            # chunk 1 matmuls inline during the ACT-idle startup;
            # its drains go to the background queue
            pj1 = stripp.tile([128, 1024], F32, tag="strip",
                              name="pj1")
            qk_chunk_mm(0, 1, 0, ND, pj1)
            for part in range(2):
                for h in range(2):
                    def c1drain(on_act=False, p=part, hh=h):
                        qk_chunk_drain(0, 1, p, pj1, on_act=on_act,
                                       hs=(hh,))
                    drain_fns[(0, 1, part, h)] = c1drain
                    bg_add(("qkd", 0, 1, part, h), 0, 400, c1drain)
            for q in range(4):
                emitted.add(("qkm", 0, 1, q))


            for g in (1, 2, 3):
                queue_dma(("xg", g), lambda g=g: load_group(g))
            for c in (2, 3):
                queue_qk(0, c)
            for ki in (2, 3, 4, 5):
                queue_v(0, ki)
            for c in (4, 5):
                queue_qk(0, c)
            for ki in (6, 7, 8, 9):
                queue_v(0, ki)
            for c in (6, 7):
                queue_qk(0, c)
            for ki in range(10, NJ):
                queue_v(0, ki)

            def need_qk(b, cs):
                force(lambda k: k[0] == "qkm" and k[1] == b
                      and k[2] in cs)
                # bulk-forced drains: ACT is idle while it waits for
                # these, so alternate them between ACT and DVE
                n = 0
                for c in cs:
                    for part in range(2):
                        for h in range(2):
                            key = ("qkd", b, c, part, h)
                            if key in emitted or key not in by_key:
                                continue
                            emitted.add(key)
                            drain_fns[(b, c, part, h)](on_act=n % 2 == 0)
                            n += 1

            # batch 0 (merged-head spans, qw <= 512)
            cr = attention_span(0, 0, 256)
            need_qk(0, (1,))
            cr = attention_span(0, 256, 256, carry=cr)
            need_qk(0, (2, 3))
            cr = attention_span(0, 512, 512, carry=cr)
            # queue b1 inputs + b1 projection work mid-flight
            queue_dma(("x1a", 0), load_b1_first)
            queue_dma(("aug1", 0), load_aug_b1)
            for g in (5, 6, 7):
                queue_dma(("xg", g), lambda g=g: load_group(g))
            for c in (0, 1, 2, 3):
                queue_qk(1, c)
            for ki in (0, 1, 2, 3):
                queue_v(1, ki)
            for c in (4, 5, 6, 7):
                queue_qk(1, c)
            for ki in range(4, NJ):
                queue_v(1, ki)
            need_qk(0, (4, 5))
            cr = attention_span(0, 1024, 512, carry=cr)
            need_qk(0, (6, 7))
            cr = attention_span(0, 1536, 512, carry=cr)

            # batch 1
            force(lambda k: k[0] in ("x1a", "aug1"))
            need_qk(1, (0, 1))
            cr = attention_span(1, 0, 512, carry=cr)
            need_qk(1, (2, 3))
            cr = attention_span(1, 512, 512, carry=cr)
            need_qk(1, (4, 5))
            cr = attention_span(1, 1024, 512, carry=cr)
            need_qk(1, (6, 7))
            cr = attention_span(1, 1536, 512, last_span=True, carry=cr)
            cr()
            force(lambda k: True)

    nc.compile()
    return nc


def _get_program():
    if "nc" not in _CACHE:
        _CACHE["nc"] = _build_program()
    return _CACHE["nc"]


def _prep_inputs(input, attention_mask, W_Q, W_K, W_V):
    f16 = np.float16

    x = np.asarray(input, dtype=np.float32).reshape(T, D)
    xT = np.ascontiguousarray(x.T).astype(f16)             # [D, T]
    # [d, p, g, tloc]
    xr = xT.reshape(ND, 128, B * 4, 512).transpose(1, 2, 0, 3)
    # xr: [p, g, d, tloc]
    xh = {}
    for b in range(B):
        g0 = xr[:, b * 4]                                    # [p, d, 512]
        xh[f"xh0a{b}"] = np.ascontiguousarray(
            g0[:, :, 0:256].reshape(128, ND * 256))
        xh[f"xh0b{b}"] = np.ascontiguousarray(
            g0[:, :, 256:512].reshape(128, ND * 256))
    rest = np.concatenate([xr[:, g] for g in (1, 2, 3, 5, 6, 7)],
                          axis=1)                            # [p, 6*d, 512]
    xhr = np.ascontiguousarray(rest.reshape(128, 6 * ND * 512))

    import ml_dtypes
    mask = np.asarray(attention_mask).astype(np.float32).reshape(1, T)
    kaug = ((mask - 1.0) * NEG_BIG).astype(f16)
    qi = np.arange(128)
    trilb = np.where(qi[None, :] >= qi[:, None], 1.0,
                     0.0).astype(ml_dtypes.bfloat16)   # keep[k,q]: q >= k
    identf = np.eye(128, dtype=np.float32)

    common = {
        **xh, "xhr": xhr, "kaug": kaug, "trilb": trilb,
        "identf": identf,
    }

    def pack_w(Wm, sl):
        wc = np.asarray(Wm, np.float32)[sl, :].astype(f16)  # [128e, D]
        return wc.reshape(128, ND, 128).transpose(2, 1, 0)   # [p, d, e]

    in_maps = []
    for c in range(N_CORES):
        sl = slice(c * E, (c + 1) * E)
        wq = pack_w(W_Q, sl)
        wk = pack_w(W_K, sl)
        wv = pack_w(W_V, sl)
        in_maps.append({
            **common,
            "wqk": np.ascontiguousarray(
                np.concatenate([wq, wk], axis=1).reshape(128, 2 * ND * 128)),
            "wv": np.ascontiguousarray(wv.reshape(128, ND * 128)),
        })
    return in_maps


def kernel(input, attention_mask, W_Q, W_K, W_V):
    from concourse.bass_utils import run_bass_kernel_spmd

    nc = _get_program()
    in_maps = _prep_inputs(input, attention_mask, W_Q, W_K, W_V)
    res = run_bass_kernel_spmd(nc, in_maps, list(range(N_CORES)))
    return np.concatenate([res.results[c]["out"] for c in range(N_CORES)],
                          axis=2)
